# revision 1
# baseline (speedup 1.0000x reference)
"""Trainium2 Bass kernel for nn_DAG_72782515798738.

Math: node j (of M=1280) computes h_j = tanh(b_j + sum_{k<IN+j} W[j,k]*state_k)
over states = [x (IN=1024), h (M)], batch B=8192. Output y = sigmoid(h[HID:]).

Strategy: data-parallel over batch (8 cores x 1024 rows). Per core, the node
recurrence is solved block-by-block (10 blocks of 128 nodes) with a fixed-point
iteration h <- tanh(p + L_diag @ h) in NODE-MAJOR layout ([node, batch] tiles):
L is strictly triangular so the iteration converges superlinearly (error drops
~10x/iteration; ~9 iterations reach fp32 accuracy). Cross-block/input terms p
are accumulated in PSUM by the tensor engine (contraction 128/1024), prefetched
during the previous block's iterations. No transposes, no per-node ops.
"""
import numpy as np

import concourse.bass as bass
import concourse.mybir as mybir
from concourse.tile import TileContext
from concourse.vector_clock import ScopedClock
from concourse.bass_utils import run_bass_kernel_spmd

F32 = mybir.dt.float32
F32R = mybir.dt.float32r   # single-pass fp32 matmul (tf32-class precision, ~3x faster)
AF = mybir.ActivationFunctionType
ALU = mybir.AluOpType

IN, HID, OUT = 1024, 1024, 256
M = HID + OUT          # 1280 computed nodes
B = 8192
NCORES = 8
BC = B // NCORES       # 1024 batch rows per core
K = 128                # node block size
NB = M // K            # 10 blocks
NIT_R = 4              # float32r updates after h0 (truncation ~5e-5 < f32r noise)
NIT_F = 0              # closing full-fp32 updates (needs fp32 operand tiles; off)
HALF = BC // 2         # 512


_wsplit_ctr = [0]


class _TileContextFix(TileContext):
    """This walrus build accepts only ONE embedded sem-wait per instruction;
    split extra waits onto single-wait NOPs, and same for the exit drain."""

    def _add_instruction(self, inst):
        si = getattr(inst, "sync_info", None)
        if si is not None and si.on_wait is not None and len(si.on_wait) > 1:
            waits = list(si.on_wait)
            for w in waits[:-1]:
                _wsplit_ctr[0] += 1
                nop = mybir.InstNoOp(name=f"wsplit_{_wsplit_ctr[0]}", ins=[], outs=[])
                nop.engine = inst.engine
                nop.sync_info = mybir.SyncInfo(on_wait=[w], on_update=[])
                super()._add_instruction(nop)
            si.on_wait = waits[-1:]
        super()._add_instruction(inst)

    def _drain_and_barrier(self, tick_clock, wait_clock):
        nc = self.nc
        probe = nc.sync.nop(nofuse=True, hint="exit_wait_carrier")
        wait_clock.add_sem_waits(probe.ins, ScopedClock({None: tick_clock.global_clock}))
        si = probe.ins.sync_info
        waits = list(si.on_wait) if si is not None and si.on_wait else []
        if len(waits) > 1:
            si.on_wait = waits[:1]
            for w in waits[1:]:
                n2 = nc.sync.nop(nofuse=True, hint="exit_wait_carrier")
                if n2.ins.sync_info is None:
                    n2.ins.sync_info = mybir.SyncInfo(on_wait=[w], on_update=[])
                else:
                    n2.ins.sync_info.on_wait = [w]
        nc.sync.drain()
        nc.all_engine_barrier()
        assert self.sems is not None
        popped = nc._tile_sem_poison_stack.pop()
        assert popped is self._sem_poison
        nc.clear_and_free_semaphores(list(self.sems.allocated().values()))
        nc.all_engine_barrier()


def _build():
    nc = bass.Bass("TRN2", target_bir_lowering=False, debug=False, num_devices=NCORES)

    xT = nc.dram_tensor("xT", [IN, BC], F32R, kind="ExternalInput")
    WxT = nc.dram_tensor("WxT", [IN, M], F32R, kind="ExternalInput")
    LTd = nc.dram_tensor("LT", [M, M], F32R, kind="ExternalInput")
    bd = nc.dram_tensor("bvec", [M, 1], F32, kind="ExternalInput")
    yT = nc.dram_tensor("yT", [OUT, BC], F32, kind="ExternalOutput")

    KT = IN // 128  # 8 contraction tiles for the input matmul

    with _TileContextFix(nc) as tc:
        with (
            tc.tile_pool(name="sb", bufs=1) as sb,
            tc.tile_pool(name="ps", bufs=1, space="PSUM") as ps,
        ):
            # persistent SBUF tiles
            xt = [sb.tile([128, BC], F32R, name=f"xt{t}", tag=f"xt{t}") for t in range(KT)]
            wx = [sb.tile([128, M], F32R, name=f"wx{t}", tag=f"wx{t}") for t in range(KT)]
            # lt[i]: rows = nodes [128i,128i+128), cols = targets [128i, 1280)
            lt = [sb.tile([128, M - 128 * i], F32R, name=f"lt{i}", tag=f"lt{i}") for i in range(NB)]
            hb = [sb.tile([128, BC], F32R, name=f"h{u}", tag=f"h{u}") for u in range(NB)]
            bt = sb.tile([128, NB], F32, name="bt", tag="bt")

            # DMAs spread across engine SWDGE/HWDGE queues, ordered so block 0
            # (then 1, 2, ...) can start as early as possible: first-half xt
            # and the first wx column-block gate p0; the rest streams behind.
            nc.gpsimd.dma_start(out=bt[:], in_=bd.ap().rearrange("(u p) o -> p (u o)", p=128))
            for t in range(KT):
                nc.scalar.dma_start(out=xt[t][:, 0:HALF], in_=xT.ap()[128 * t:128 * (t + 1), 0:HALF])
                nc.gpsimd.dma_start(out=wx[t][:, 0:128], in_=WxT.ap()[128 * t:128 * (t + 1), 0:128])
            for t in range(KT):
                nc.scalar.dma_start(out=xt[t][:, HALF:], in_=xT.ap()[128 * t:128 * (t + 1), HALF:])
                nc.gpsimd.dma_start(out=wx[t][:, 128:256], in_=WxT.ap()[128 * t:128 * (t + 1), 128:256])
            nc.sync.dma_start(out=lt[0][:], in_=LTd.ap()[0:128, 0:])
            for t in range(KT):
                nc.gpsimd.dma_start(out=wx[t][:, 256:], in_=WxT.ap()[128 * t:128 * (t + 1), 256:])
            for i in range(1, NB):
                nc.sync.dma_start(
                    out=lt[i][:], in_=LTd.ap()[128 * i:128 * (i + 1), 128 * i:])

            with (
                tc.tile_pool(name="pp_pool", bufs=2, space="PSUM") as pp_pool,
                tc.tile_pool(name="lh_pool", bufs=2, space="PSUM") as lh_pool,
                tc.tile_pool(name="psb_pool", bufs=2) as psb_pool,
                tc.tile_pool(name="tin_pool", bufs=2, space="PSUM") as tin_pool,
                tc.tile_pool(name="y_pool", bufs=2) as y_pool,
            ):
                def prefetch_input(u, p_ps):
                    """input-matmul contributions to p for block u (start of group)"""
                    for h in range(2):
                        sl = slice(HALF * h, HALF * (h + 1))
                        for t in range(KT):
                            nc.tensor.matmul(
                                p_ps[:, sl],
                                wx[t][:, 128 * u:128 * (u + 1)],
                                xt[t][:, sl],
                                start=(t == 0), stop=False)

                def prefetch_cross(u, p_ps, i, last):
                    """contribution of completed block i (<u) to p of block u"""
                    for h in range(2):
                        sl = slice(HALF * h, HALF * (h + 1))
                        nc.tensor.matmul(
                            p_ps[:, sl],
                            lt[i][:, 128 * (u - i):128 * (u - i + 1)],
                            hb[i][:, sl],
                            start=False, stop=last)

                p_cur = pp_pool.tile([128, BC], F32, name="pp", tag="pp")
                prefetch_input(0, p_cur)

                for u in range(NB):
                    bcol = bt[:, u:u + 1]
                    # h0 = tanh(p + b) straight from PSUM (ACT)
                    for h in range(2):
                        sl = slice(HALF * h, HALF * (h + 1))
                        nc.scalar.activation(hb[u][:, sl], p_cur[:, sl], AF.Tanh, bias=bcol)
                    # p -> SBUF copy (DVE), frees the PSUM accumulator for prefetch
                    p_sb = psb_pool.tile([128, BC], F32, name="psb", tag="psb")
                    for h in range(2):
                        sl = slice(HALF * h, HALF * (h + 1))
                        nc.vector.tensor_copy(p_sb[:, sl], p_cur[:, sl])

                    # fixed-point iterations, two independent batch halves.
                    # NIT_R fp32r rounds converge to ~3e-5, then NIT_F full-fp32
                    # rounds restore fp32-class accuracy.
                    for k in range(NIT_R + NIT_F):
                        for h in range(2):
                            sl = slice(HALF * h, HALF * (h + 1))
                            lh = lh_pool.tile([128, HALF], F32, name="lh", tag="lh")
                            nc.tensor.matmul(
                                lh[:], lt[u][:, 0:128], hb[u][:, sl],
                                start=True, stop=True)
                            tin = tin_pool.tile([128, HALF], F32, name="tin", tag="tin")
                            nc.vector.tensor_tensor(
                                out=tin[:], in0=lh[:], in1=p_sb[:, sl], op=ALU.add)
                            nc.scalar.activation(hb[u][:, sl], tin[:], AF.Tanh, bias=bcol)

                    # prefetch next block's p (fills PE gaps in the iteration
                    # chain): input + cross from blocks <= u; the i=u pair last.
                    if u + 1 < NB:
                        p_nxt = pp_pool.tile([128, BC], F32, name="pp", tag="pp")
                        prefetch_input(u + 1, p_nxt)
                        for i in range(u):
                            prefetch_cross(u + 1, p_nxt, i, last=False)
                        prefetch_cross(u + 1, p_nxt, u, last=True)
                        p_cur = p_nxt

                    # output blocks: y = sigmoid(h), DMA out
                    if u >= NB - 2:
                        yt = y_pool.tile([128, BC], F32, name="y", tag="y")
                        for h in range(2):
                            sl = slice(HALF * h, HALF * (h + 1))
                            nc.scalar.activation(yt[:, sl], hb[u][:, sl], AF.Sigmoid)
                        r0 = 128 * (u - (NB - 2))
                        nc.gpsimd.dma_start(out=yT.ap()[r0:r0 + 128, :], in_=yt[:])
    return nc


def _enable_ldw_opt():
    """Walrus disables its LDWEIGHTS dedup by default; enabling it shaves a
    few percent here (verified correct for this kernel)."""
    import concourse.bass_utils as _bu
    if getattr(_bu.run_command, "_ldw_patched", False):
        return
    _orig = _bu.run_command

    def _patched(argv, **kw):
        try:
            argv = ["--enable-ldw-opt=true" if a == "--enable-ldw-opt=false" else a
                    for a in argv]
        except Exception:
            pass
        return _orig(argv, **kw)

    _patched._ldw_patched = True
    _bu.run_command = _patched


_nc_cache = None


def kernel(x, W, b):
    global _nc_cache
    x = np.asarray(x, dtype=np.float32)
    W = np.asarray(W, dtype=np.float32)
    b = np.asarray(b, dtype=np.float32)

    xT = np.ascontiguousarray(x.T)                       # [IN, B]
    WxT = np.ascontiguousarray(W[:, :IN].T)              # [IN, M]
    LT = np.ascontiguousarray(np.triu(W[:, IN:].T, 1))   # [M, M], LT[i,j]=W[j,IN+i], i<j
    b2 = np.ascontiguousarray(b.reshape(M, 1))

    if _nc_cache is None:
        _enable_ldw_opt()
        _nc_cache = _build()

    in_maps = [
        {"xT": np.ascontiguousarray(xT[:, c * BC:(c + 1) * BC]),
         "WxT": WxT, "LT": LT, "bvec": b2}
        for c in range(NCORES)
    ]
    res = run_bass_kernel_spmd(_nc_cache, in_maps, list(range(NCORES)))
    y = np.concatenate(
        [np.ascontiguousarray(res.results[c]["yT"].T) for c in range(NCORES)], axis=0)
    return y



# revision 6
# speedup vs baseline: 1.6838x; 1.6838x over previous
"""Trainium2 Bass kernel for nn_DAG_72782515798738.

Math: node j (of M=1280) computes h_j = tanh(b_j + sum_{k<IN+j} W[j,k]*state_k)
over states = [x (IN=1024), h (M)], batch B=8192. Output y = sigmoid(h[HID:]).

Strategy: data-parallel over batch (8 cores x 1024 rows). Per core, the node
recurrence is solved block-by-block (10 blocks of 128 nodes, NODE-major tiles
[node, batch]). All matmuls run in bf16 (PSUM accumulates fp32); the 2e-2
correctness gate leaves ample room (measured ~7e-3). Per block the
pre-activation accumulator t lives in PSUM: prefetched input/cross matmuls
build t = p, then two fixed-point refinements accumulate in place:
  h0 = tanh(t+b);  t += Ld@h0;  h1 = tanh(t+b);  t += Ld@(h1-h0);  h2 = tanh(t+b)
so the only non-PE work on the chain is the tanh (ACT) and one bf16 DVE sub.
Next-block prefetch matmuls are issued ahead of the iteration matmuls so the
PE FIFO never stalls on the activation chain.
"""
import numpy as np
import ml_dtypes

import concourse.bass as bass
import concourse.mybir as mybir
from concourse.tile import TileContext
from concourse.vector_clock import ScopedClock
from concourse.bass_utils import run_bass_kernel_spmd

F32 = mybir.dt.float32
BF16 = mybir.dt.bfloat16
AF = mybir.ActivationFunctionType
ALU = mybir.AluOpType

IN, HID, OUT = 1024, 1024, 256
M = HID + OUT          # 1280 computed nodes
B = 8192
NCORES = 8
BC = B // NCORES       # 1024 batch rows per core
K = 128                # node block size
NB = M // K            # 10 blocks
KT = IN // 128         # 8 contraction tiles for the input matmul
HALF = BC // 2         # 512

# lt packing offsets: block i occupies cols [LOFF[i], LOFF[i] + M - 128*i)
LOFF = [0]
for _i in range(1, NB):
    LOFF.append(LOFF[-1] + (M - 128 * (_i - 1)))
LTOT = LOFF[-1] + (M - 128 * (NB - 1))  # 7040

_wsplit_ctr = [0]


class _TileContextFix(TileContext):
    """This walrus build accepts only ONE embedded sem-wait per instruction;
    split extra waits onto single-wait NOPs, and same for the exit drain."""

    def _add_instruction(self, inst):
        si = getattr(inst, "sync_info", None)
        if si is not None and si.on_wait is not None and len(si.on_wait) > 1:
            waits = list(si.on_wait)
            for w in waits[:-1]:
                _wsplit_ctr[0] += 1
                nop = mybir.InstNoOp(name=f"wsplit_{_wsplit_ctr[0]}", ins=[], outs=[])
                nop.engine = inst.engine
                nop.sync_info = mybir.SyncInfo(on_wait=[w], on_update=[])
                super()._add_instruction(nop)
            si.on_wait = waits[-1:]
        super()._add_instruction(inst)

    def _drain_and_barrier(self, tick_clock, wait_clock):
        nc = self.nc
        probe = nc.sync.nop(nofuse=True, hint="exit_wait_carrier")
        wait_clock.add_sem_waits(probe.ins, ScopedClock({None: tick_clock.global_clock}))
        si = probe.ins.sync_info
        waits = list(si.on_wait) if si is not None and si.on_wait else []
        if len(waits) > 1:
            si.on_wait = waits[:1]
            for w in waits[1:]:
                n2 = nc.sync.nop(nofuse=True, hint="exit_wait_carrier")
                if n2.ins.sync_info is None:
                    n2.ins.sync_info = mybir.SyncInfo(on_wait=[w], on_update=[])
                else:
                    n2.ins.sync_info.on_wait = [w]
        nc.sync.drain()
        nc.all_engine_barrier()
        assert self.sems is not None
        popped = nc._tile_sem_poison_stack.pop()
        assert popped is self._sem_poison
        nc.clear_and_free_semaphores(list(self.sems.allocated().values()))
        nc.all_engine_barrier()


def _build():
    nc = bass.Bass("TRN2", target_bir_lowering=False, debug=False, num_devices=NCORES)

    xT = nc.dram_tensor("xT", [IN, BC], BF16, kind="ExternalInput")
    WxT = nc.dram_tensor("WxT", [IN, M], BF16, kind="ExternalInput")
    LTd = nc.dram_tensor("LT", [M, M], BF16, kind="ExternalInput")
    bd = nc.dram_tensor("bvec", [M, 1], F32, kind="ExternalInput")
    yT = nc.dram_tensor("yT", [OUT, BC], F32, kind="ExternalOutput")

    with _TileContextFix(nc) as tc:
        with (
            tc.tile_pool(name="sb", bufs=1) as sb,
        ):
            # persistent SBUF tiles
            # xt: half-major then ktile: half h at cols 4096h, ktile t at +512t
            xt = sb.tile([128, 2 * KT * HALF], BF16, name="xt", tag="xt")
            # wx groups: g0 = block 0 (ktile t at 128t), g1 = blocks 1-4
            # (ktile t at 1024+512t), g2 = blocks 5-9 (ktile t at 5120+640t)
            wx = sb.tile([128, KT * M], BF16, name="wx", tag="wx")
            # lt: block i rows at cols LOFF[i], width M-128i
            lt = sb.tile([128, LTOT], BF16, name="lt", tag="lt")
            # hb: final h per block u at cols 1024u
            hb = sb.tile([128, NB * BC], BF16, name="hb", tag="hb")
            bt = sb.tile([128, NB], F32, name="bt", tag="bt")

            def wx_ap(t, u):
                if u == 0:
                    c = 128 * t
                elif u <= 4:
                    c = 1024 + 512 * t + 128 * (u - 1)
                else:
                    c = 5120 + 640 * t + 128 * (u - 5)
                return wx[:, c:c + 128]

            def xt_ap(t, h):
                c = 4096 * h + 512 * t
                return xt[:, c:c + 512]

            def ltd_ap(u):
                return lt[:, LOFF[u]:LOFF[u] + 128]

            def ltx_ap(u, i):
                c = LOFF[i] + 128 * (u - i)
                return lt[:, c:c + 128]

            # ---- DMA in (engines: sync/gpsimd/vector; keep Scalar pure ACT) ----
            nc.sync.dma_start(out=bt[:], in_=bd.ap().rearrange("(u p) o -> p (u o)", p=128))
            nc.gpsimd.dma_start(
                out=wx[:, 0:1024].rearrange("p (t c) -> p t c", t=KT),
                in_=WxT.ap()[:, 0:128].rearrange("(t p) c -> p t c", p=128))
            nc.sync.dma_start(
                out=xt[:, 0:4096].rearrange("p (t c) -> p t c", t=KT),
                in_=xT.ap()[:, 0:HALF].rearrange("(t p) c -> p t c", p=128))
            nc.gpsimd.dma_start(
                out=xt[:, 4096:8192].rearrange("p (t c) -> p t c", t=KT),
                in_=xT.ap()[:, HALF:].rearrange("(t p) c -> p t c", p=128))
            nc.scalar.dma_start(
                out=lt[:, LOFF[0]:LOFF[0] + M], in_=LTd.ap()[0:128, 0:])
            nc.sync.dma_start(
                out=wx[:, 1024:5120].rearrange("p (t c) -> p t c", t=KT),
                in_=WxT.ap()[:, 128:640].rearrange("(t p) c -> p t c", p=128))
            nc.scalar.dma_start(
                out=lt[:, LOFF[1]:LOFF[1] + M - 128], in_=LTd.ap()[128:256, 128:])
            nc.gpsimd.dma_start(
                out=wx[:, 5120:10240].rearrange("p (t c) -> p t c", t=KT),
                in_=WxT.ap()[:, 640:].rearrange("(t p) c -> p t c", p=128))
            for i in range(2, NB):
                eng = (nc.scalar, nc.sync, nc.gpsimd)[i % 3]
                eng.dma_start(
                    out=lt[:, LOFF[i]:LOFF[i] + M - 128 * i],
                    in_=LTd.ap()[128 * i:128 * (i + 1), 128 * i:])

            with (
                tc.tile_pool(name="pp_pool", bufs=3, space="PSUM") as pp_pool,
                tc.tile_pool(name="ht0_pool", bufs=2) as ht0_pool,
                tc.tile_pool(name="ht1_pool", bufs=2) as ht1_pool,
                tc.tile_pool(name="dt_pool", bufs=2) as dt_pool,
                tc.tile_pool(name="y_pool", bufs=2) as y_pool,
            ):
                def prefetch_input(u, t_ps):
                    """input-matmul contributions to t for block u; ktile-outer
                    so walrus dedups LDWEIGHTS across the half pair."""
                    for t in range(KT):
                        for h in range(2):
                            sl = slice(HALF * h, HALF * (h + 1))
                            nc.tensor.matmul(
                                t_ps[:, sl], wx_ap(t, u), xt_ap(t, h),
                                start=(t == 0), stop=False)

                def prefetch_cross(u, t_ps, i, last):
                    """contribution of completed block i (<u) to t of block u"""
                    for h in range(2):
                        sl = slice(HALF * h, HALF * (h + 1))
                        nc.tensor.matmul(
                            t_ps[:, sl], ltx_ap(u, i), hb[:, BC * i + HALF * h:BC * i + HALF * (h + 1)],
                            start=False, stop=(last and h == 1))

                t_cur = pp_pool.tile([128, BC], F32, name="pp", tag="pp")
                prefetch_input(0, t_cur)

                for u in range(NB):
                    bcol = bt[:, u:u + 1]

                    # A. bulk prefetch for u+1 (keeps the PE FIFO fed while
                    # this block's activation chain runs)
                    t_nxt = None
                    if u + 1 < NB:
                        t_nxt = pp_pool.tile([128, BC], F32, name="pp", tag="pp")
                        prefetch_input(u + 1, t_nxt)
                        for i in range(u):
                            prefetch_cross(u + 1, t_nxt, i, last=False)

                    # B. h0 = tanh(t + b)
                    ht0 = ht0_pool.tile([128, BC], BF16, name="ht0", tag="ht0")
                    for h in range(2):
                        sl = slice(HALF * h, HALF * (h + 1))
                        nc.scalar.activation(ht0[:, sl], t_cur[:, sl], AF.Tanh, bias=bcol)

                    # C. t += Ld @ h0
                    for h in range(2):
                        sl = slice(HALF * h, HALF * (h + 1))
                        nc.tensor.matmul(
                            t_cur[:, sl], ltd_ap(u), ht0[:, sl],
                            start=False, stop=False, skip_group_check=True)

                    # D. h1 = tanh(t + b)
                    ht1 = ht1_pool.tile([128, BC], BF16, name="ht1", tag="ht1")
                    for h in range(2):
                        sl = slice(HALF * h, HALF * (h + 1))
                        nc.scalar.activation(ht1[:, sl], t_cur[:, sl], AF.Tanh, bias=bcol)

                    # E. d = h1 - h0 (bf16 DVE)
                    dt = dt_pool.tile([128, BC], BF16, name="dt", tag="dt")
                    for h in range(2):
                        sl = slice(HALF * h, HALF * (h + 1))
                        nc.vector.tensor_tensor(
                            out=dt[:, sl], in0=ht1[:, sl], in1=ht0[:, sl], op=ALU.subtract)

                    # F. t += Ld @ d
                    for h in range(2):
                        sl = slice(HALF * h, HALF * (h + 1))
                        nc.tensor.matmul(
                            t_cur[:, sl], ltd_ap(u), dt[:, sl],
                            start=False, stop=False, skip_group_check=True)

                    # G. h2 = tanh(t + b) -> final
                    for h in range(2):
                        sl = slice(HALF * h, HALF * (h + 1))
                        nc.scalar.activation(
                            hb[:, BC * u + HALF * h:BC * u + HALF * (h + 1)],
                            t_cur[:, sl], AF.Tanh, bias=bcol)

                    # H. adjacent cross u+1 <- u
                    if u + 1 < NB:
                        prefetch_cross(u + 1, t_nxt, u, last=True)
                        t_cur = t_nxt

                # outputs: y = sigmoid(h) for blocks NB-2, NB-1
                for j, ub in enumerate((NB - 2, NB - 1)):
                    yt = y_pool.tile([128, BC], F32, name="y", tag="y")
                    nc.scalar.activation(yt[:], hb[:, BC * ub:BC * (ub + 1)], AF.Sigmoid)
                    nc.gpsimd.dma_start(out=yT.ap()[128 * j:128 * (j + 1), :], in_=yt[:])
    return nc


_nc_cache = None
BF16NP = ml_dtypes.bfloat16


def _prep(x, W, b):
    x = np.asarray(x, dtype=np.float32)
    W = np.asarray(W, dtype=np.float32)
    b = np.asarray(b, dtype=np.float32)
    xT = np.ascontiguousarray(x.T.astype(BF16NP))               # [IN, B] bf16
    WxT = np.ascontiguousarray(W[:, :IN].T.astype(BF16NP))      # [IN, M] bf16
    LT = np.ascontiguousarray(np.triu(W[:, IN:].T, 1).astype(BF16NP))  # [M, M] bf16
    b2 = np.ascontiguousarray(b.reshape(M, 1))
    return xT, WxT, LT, b2


def kernel(x, W, b):
    global _nc_cache
    xT, WxT, LT, b2 = _prep(x, W, b)

    if _nc_cache is None:
        _nc_cache = _build()

    in_maps = [
        {"xT": np.ascontiguousarray(xT[:, c * BC:(c + 1) * BC]),
         "WxT": WxT, "LT": LT, "bvec": b2}
        for c in range(NCORES)
    ]
    res = run_bass_kernel_spmd(_nc_cache, in_maps, list(range(NCORES)))
    y = np.concatenate(
        [np.ascontiguousarray(res.results[c]["yT"].T) for c in range(NCORES)], axis=0)
    return y


# revision 9
# speedup vs baseline: 1.8360x; 1.0903x over previous
"""Trainium2 Bass kernel for nn_DAG_72782515798738.

Math: node j (of M=1280) computes h_j = tanh(b_j + sum_{k<IN+j} W[j,k]*state_k)
over states = [x (IN=1024), h (M)], batch B=8192. Output y = sigmoid(h[HID:]).

Strategy: data-parallel over batch (8 cores x 1024 rows). Per core, the node
recurrence is solved block-by-block (10 blocks of 128 nodes, NODE-major tiles
[node, batch]). All matmuls run in bf16 (PSUM accumulates fp32); the 2e-2
correctness gate leaves ample room (measured ~6e-3). Per block the
pre-activation accumulator t lives in PSUM: prefetched input/cross matmuls
build t = p, then two fixed-point refinements accumulate in place:
  h0 = tanh(t+b);  t += Ld@h0;  h1 = tanh(t+b);  t += Ld@(h1-h0);  h2 = tanh(t+b)
so the only non-PE work on the chain is the tanh (ACT) and one bf16 DVE sub.

All inputs are pre-rearranged on the host into the exact SBUF tile layouts so
every DMA is a fully-contiguous col-slice copy (8-16KB per partition line).
Prefetch matmuls for block u+1 are interleaved around the iteration matmuls
of block u so the PE FIFO never stalls on the activation chain.
"""
import numpy as np
import ml_dtypes

import concourse.bass as bass
import concourse.mybir as mybir
from concourse.tile import TileContext
from concourse.vector_clock import ScopedClock
from concourse.bass_utils import run_bass_kernel_spmd

F32 = mybir.dt.float32
BF16 = mybir.dt.bfloat16
AF = mybir.ActivationFunctionType
ALU = mybir.AluOpType

IN, HID, OUT = 1024, 1024, 256
M = HID + OUT          # 1280 computed nodes
B = 8192
NCORES = 8
BC = B // NCORES       # 1024 batch rows per core
K = 128                # node block size
NB = M // K            # 10 blocks
KT = IN // 128         # 8 contraction tiles for the input matmul
HALF = BC // 2         # 512

# lt packing offsets: block i occupies cols [LOFF[i], LOFF[i] + M - 128*i)
LOFF = [0]
for _i in range(1, NB):
    LOFF.append(LOFF[-1] + (M - 128 * (_i - 1)))
LTOT = LOFF[-1] + (M - 128 * (NB - 1))  # 7040

# wx packing: group g0 = block 0 (ktile t at 128t), g1 = blocks 1-4
# (ktile t at 1024 + 512t), g2 = blocks 5-9 (ktile t at 5120 + 640t)
WXTOT = KT * M  # 10240

_wsplit_ctr = [0]


class _TileContextFix(TileContext):
    """This walrus build accepts only ONE embedded sem-wait per instruction;
    split extra waits onto single-wait NOPs, and same for the exit drain."""

    def _add_instruction(self, inst):
        si = getattr(inst, "sync_info", None)
        if si is not None and si.on_wait is not None and len(si.on_wait) > 1:
            waits = list(si.on_wait)
            for w in waits[:-1]:
                _wsplit_ctr[0] += 1
                nop = mybir.InstNoOp(name=f"wsplit_{_wsplit_ctr[0]}", ins=[], outs=[])
                nop.engine = inst.engine
                nop.sync_info = mybir.SyncInfo(on_wait=[w], on_update=[])
                super()._add_instruction(nop)
            si.on_wait = waits[-1:]
        super()._add_instruction(inst)

    def _drain_and_barrier(self, tick_clock, wait_clock):
        nc = self.nc
        probe = nc.sync.nop(nofuse=True, hint="exit_wait_carrier")
        wait_clock.add_sem_waits(probe.ins, ScopedClock({None: tick_clock.global_clock}))
        si = probe.ins.sync_info
        waits = list(si.on_wait) if si is not None and si.on_wait else []
        if len(waits) > 1:
            si.on_wait = waits[:1]
            for w in waits[1:]:
                n2 = nc.sync.nop(nofuse=True, hint="exit_wait_carrier")
                if n2.ins.sync_info is None:
                    n2.ins.sync_info = mybir.SyncInfo(on_wait=[w], on_update=[])
                else:
                    n2.ins.sync_info.on_wait = [w]
        nc.sync.drain()
        nc.all_engine_barrier()
        assert self.sems is not None
        popped = nc._tile_sem_poison_stack.pop()
        assert popped is self._sem_poison
        nc.clear_and_free_semaphores(list(self.sems.allocated().values()))
        nc.all_engine_barrier()


def _build():
    nc = bass.Bass("TRN2", target_bir_lowering=False, debug=False, num_devices=NCORES)

    # host-rearranged inputs, already in SBUF tile layout
    xtr = nc.dram_tensor("xtr", [128, 2 * KT * HALF], BF16, kind="ExternalInput")
    wxr = nc.dram_tensor("wxr", [128, WXTOT], BF16, kind="ExternalInput")
    ltr = nc.dram_tensor("ltr", [128, LTOT], BF16, kind="ExternalInput")
    btr = nc.dram_tensor("btr", [128, NB], F32, kind="ExternalInput")
    yT = nc.dram_tensor("yT", [OUT, BC], F32, kind="ExternalOutput")

    with _TileContextFix(nc) as tc:
        with (
            tc.tile_pool(name="sb", bufs=1) as sb,
        ):
            # persistent SBUF tiles (same layouts as the dram tensors)
            xt = sb.tile([128, 2 * KT * HALF], BF16, name="xt", tag="xt")
            wx = sb.tile([128, WXTOT], BF16, name="wx", tag="wx")
            lt = sb.tile([128, LTOT], BF16, name="lt", tag="lt")
            hb = sb.tile([128, NB * BC], BF16, name="hb", tag="hb")
            bt = sb.tile([128, NB], F32, name="bt", tag="bt")

            def wx_ap(t, u):
                if u == 0:
                    c = 128 * t
                elif u <= 4:
                    c = 1024 + 512 * t + 128 * (u - 1)
                else:
                    c = 5120 + 640 * t + 128 * (u - 5)
                return wx[:, c:c + 128]

            def xt_ap(t, h):
                c = 4096 * h + 512 * t
                return xt[:, c:c + 512]

            def ltd_ap(u):
                return lt[:, LOFF[u]:LOFF[u] + 128]

            def ltx_ap(u, i):
                c = LOFF[i] + 128 * (u - i)
                return lt[:, c:c + 128]

            # ---- DMA in: contiguous col-slice copies, big partition lines.
            # Order: what block 0 needs first; spread across sync/gpsimd
            # queues; scalar only issues before its ACT stream begins.
            nc.scalar.dma_start(out=bt[:], in_=btr.ap()[:, :])
            nc.gpsimd.dma_start(out=wx[:, 0:1024], in_=wxr.ap()[:, 0:1024])
            nc.sync.dma_start(out=xt[:, 0:4096], in_=xtr.ap()[:, 0:4096])
            nc.gpsimd.dma_start(out=xt[:, 4096:8192], in_=xtr.ap()[:, 4096:8192])
            nc.scalar.dma_start(
                out=lt[:, LOFF[0]:LOFF[2]], in_=ltr.ap()[:, LOFF[0]:LOFF[2]])
            nc.sync.dma_start(out=wx[:, 1024:5120], in_=wxr.ap()[:, 1024:5120])
            nc.scalar.dma_start(
                out=lt[:, LOFF[2]:LOFF[5]], in_=ltr.ap()[:, LOFF[2]:LOFF[5]])
            nc.gpsimd.dma_start(out=wx[:, 5120:10240], in_=wxr.ap()[:, 5120:10240])
            nc.sync.dma_start(
                out=lt[:, LOFF[5]:LTOT], in_=ltr.ap()[:, LOFF[5]:LTOT])

            with (
                tc.tile_pool(name="pp_pool", bufs=3, space="PSUM") as pp_pool,
                tc.tile_pool(name="ht0_pool", bufs=2) as ht0_pool,
                tc.tile_pool(name="ht1_pool", bufs=2) as ht1_pool,
                tc.tile_pool(name="dt_pool", bufs=2) as dt_pool,
                tc.tile_pool(name="y_pool", bufs=2) as y_pool,
            ):
                def mm_input(u, t_ps, t, h, start):
                    sl = slice(HALF * h, HALF * (h + 1))
                    nc.tensor.matmul(
                        t_ps[:, sl], wx_ap(t, u), xt_ap(t, h),
                        start=start, stop=False)

                def mm_cross(u, t_ps, i, h, last=False):
                    sl = slice(HALF * h, HALF * (h + 1))
                    nc.tensor.matmul(
                        t_ps[:, sl], ltx_ap(u, i),
                        hb[:, BC * i + HALF * h:BC * i + HALF * (h + 1)],
                        start=False, stop=last)

                def mm_iter(u, t_ps, rhs, h):
                    sl = slice(HALF * h, HALF * (h + 1))
                    nc.tensor.matmul(
                        t_ps[:, sl], ltd_ap(u), rhs[:, sl],
                        start=False, stop=False, skip_group_check=True)

                t_cur = pp_pool.tile([128, BC], F32, name="pp", tag="pp")
                for t in range(KT):
                    for h in range(2):
                        mm_input(0, t_cur, t, h, start=(t == 0))

                yhalf = []   # deferred sigmoid work: (ub, h)

                def emit_sigmoid(ub, h, ytiles):
                    if ub not in ytiles:
                        ytiles[ub] = y_pool.tile([128, BC], F32, name="y", tag="y")
                    yt = ytiles[ub]
                    sl = slice(HALF * h, HALF * (h + 1))
                    nc.scalar.activation(
                        yt[:, sl], hb[:, BC * ub + HALF * h:BC * ub + HALF * (h + 1)],
                        AF.Sigmoid)
                    return yt

                ytiles = {}
                for u in range(NB):
                    bcol = bt[:, u:u + 1]
                    last = u + 1 >= NB

                    # prefetch MM stream for block u+1, split into chunks that
                    # sandwich the iteration matmuls (keeps PE FIFO fed while
                    # the tanh chain runs, without delaying the chain)
                    pre = []
                    t_nxt = None
                    if not last:
                        t_nxt = pp_pool.tile([128, BC], F32, name="pp", tag="pp")
                        for t in range(KT):
                            for h in range(2):
                                pre.append(("in", t, h, t == 0))
                        for i in range(u):
                            for h in range(2):
                                pre.append(("x", i, h, False))

                    def emit_pre(n):
                        for _ in range(n):
                            if not pre:
                                return
                            kind, a, h, s = pre.pop(0)
                            if kind == "in":
                                mm_input(u + 1, t_nxt, a, h, start=s)
                            else:
                                mm_cross(u + 1, t_nxt, a, h)

                    # B. h0 = tanh(t + b)
                    ht0 = ht0_pool.tile([128, BC], BF16, name="ht0", tag="ht0")
                    for h in range(2):
                        sl = slice(HALF * h, HALF * (h + 1))
                        nc.scalar.activation(ht0[:, sl], t_cur[:, sl], AF.Tanh, bias=bcol)
                    if u == NB - 1:
                        # slot block NB-2's sigmoid halves into the chain
                        # bubbles (ACT idles while the iter matmuls run)
                        emit_sigmoid(NB - 2, 0, ytiles)

                    emit_pre(6)                 # ~1.3us of PE work before iter1
                    mm_iter(u, t_cur, ht0, 0)   # C
                    mm_iter(u, t_cur, ht0, 1)

                    # D. h1 = tanh(t + b)
                    ht1 = ht1_pool.tile([128, BC], BF16, name="ht1", tag="ht1")
                    for h in range(2):
                        sl = slice(HALF * h, HALF * (h + 1))
                        nc.scalar.activation(ht1[:, sl], t_cur[:, sl], AF.Tanh, bias=bcol)
                    if u == NB - 1:
                        emit_sigmoid(NB - 2, 1, ytiles)

                    # E. d = h1 - h0 (bf16 DVE)
                    dt = dt_pool.tile([128, BC], BF16, name="dt", tag="dt")
                    for h in range(2):
                        sl = slice(HALF * h, HALF * (h + 1))
                        nc.vector.tensor_tensor(
                            out=dt[:, sl], in0=ht1[:, sl], in1=ht0[:, sl], op=ALU.subtract)

                    emit_pre(5)                 # cover the h1+sub latency
                    mm_iter(u, t_cur, dt, 0)    # F
                    mm_iter(u, t_cur, dt, 1)

                    # G. h2 = tanh(t + b) -> final
                    for h in range(2):
                        sl = slice(HALF * h, HALF * (h + 1))
                        nc.scalar.activation(
                            hb[:, BC * u + HALF * h:BC * u + HALF * (h + 1)],
                            t_cur[:, sl], AF.Tanh, bias=bcol)
                        if u == NB - 1:
                            # final outputs right behind the last tanh halves
                            emit_sigmoid(NB - 1, h, ytiles)

                    emit_pre(len(pre))          # drain remaining prefetch
                    if not last:
                        # adjacent cross u+1 <- u (needs h2)
                        for h in range(2):
                            mm_cross(u + 1, t_nxt, u, h, last=(h == 1))
                        t_cur = t_nxt

                # DMA out
                y8 = ytiles[NB - 2]
                y9 = ytiles[NB - 1]
                nc.gpsimd.dma_start(out=yT.ap()[0:128, :], in_=y8[:])
                nc.gpsimd.dma_start(out=yT.ap()[128:256, 0:HALF], in_=y9[:, 0:HALF])
                nc.gpsimd.dma_start(out=yT.ap()[128:256, HALF:], in_=y9[:, HALF:])
    return nc


_nc_cache = None
BF16NP = ml_dtypes.bfloat16


def _prep(x, W, b):
    """Rearrange full inputs into per-core SBUF-layout arrays."""
    x = np.asarray(x, dtype=np.float32)
    W = np.asarray(W, dtype=np.float32)
    b = np.asarray(b, dtype=np.float32)

    # wxr: [128, 10240] — W[:, :IN].T grouped (see _build)
    WxT = W[:, :IN].T.astype(BF16NP)                  # [IN, M]
    A = WxT.reshape(KT, 128, M)                       # [t, p, m]
    g0 = A[:, :, 0:128].transpose(1, 0, 2).reshape(128, -1)
    g1 = A[:, :, 128:640].transpose(1, 0, 2).reshape(128, -1)
    g2 = A[:, :, 640:1280].transpose(1, 0, 2).reshape(128, -1)
    wxr = np.ascontiguousarray(np.concatenate([g0, g1, g2], axis=1))

    # ltr: [128, 7040] — strictly-lower L blocks, row-block i cols [128i:M)
    LT = np.triu(W[:, IN:].T, 1).astype(BF16NP)       # [M, M]
    ltr = np.ascontiguousarray(np.concatenate(
        [LT[128 * i:128 * (i + 1), 128 * i:] for i in range(NB)], axis=1))

    # btr: [128, NB]
    btr = np.ascontiguousarray(b.reshape(NB, 128).T)

    # xtr per core: [128, 8192], cols = 4096h + 512t + c
    xb = x.astype(BF16NP)
    xtrs = []
    for c in range(NCORES):
        xTc = np.ascontiguousarray(xb[c * BC:(c + 1) * BC].T)   # [IN, BC]
        arr = xTc.reshape(KT, 128, 2, HALF)                     # [t, p, h, c]
        xtrs.append(np.ascontiguousarray(
            arr.transpose(1, 2, 0, 3).reshape(128, -1)))
    return xtrs, wxr, ltr, btr


def kernel(x, W, b):
    global _nc_cache
    xtrs, wxr, ltr, btr = _prep(x, W, b)

    if _nc_cache is None:
        _nc_cache = _build()

    in_maps = [
        {"xtr": xtrs[c], "wxr": wxr, "ltr": ltr, "btr": btr}
        for c in range(NCORES)
    ]
    res = run_bass_kernel_spmd(_nc_cache, in_maps, list(range(NCORES)))
    y = np.concatenate(
        [np.ascontiguousarray(res.results[c]["yT"].T) for c in range(NCORES)], axis=0)
    return y


# revision 13
# speedup vs baseline: 1.8713x; 1.0193x over previous
"""Trainium2 Bass kernel for nn_DAG_72782515798738.

Math: node j (of M=1280) computes h_j = tanh(b_j + sum_{k<IN+j} W[j,k]*state_k)
over states = [x (IN=1024), h (M)], batch B=8192. Output y = sigmoid(h[HID:]).

Strategy: data-parallel over batch (8 cores x 1024 rows). Per core, the node
recurrence is solved block-by-block (10 blocks of 128 nodes, NODE-major tiles
[node, batch]). All matmuls run in bf16 (PSUM accumulates fp32); the 2e-2
correctness gate leaves ample room (measured ~6e-3). Per block the
pre-activation accumulator t lives in PSUM: prefetched input/cross matmuls
build t = p, then two fixed-point refinements accumulate in place:
  h0 = tanh(t+b);  t += Ld@h0;  h1 = tanh(t+b);  t += Ld@(h1-h0);  h2 = tanh(t+b)
so the only non-PE work on the chain is the tanh (ACT) and one bf16 DVE sub.

All inputs are pre-rearranged on the host into the exact SBUF tile layouts so
every DMA is a fully-contiguous col-slice copy (8-16KB per partition line).
Prefetch matmuls for block u+1 are interleaved around the iteration matmuls
of block u so the PE FIFO never stalls on the activation chain.
"""
import numpy as np
import ml_dtypes

import concourse.bass as bass
import concourse.mybir as mybir
from concourse.tile import TileContext
from concourse.vector_clock import ScopedClock
from concourse.bass_utils import run_bass_kernel_spmd

F32 = mybir.dt.float32
BF16 = mybir.dt.bfloat16
AF = mybir.ActivationFunctionType
ALU = mybir.AluOpType

IN, HID, OUT = 1024, 1024, 256
M = HID + OUT          # 1280 computed nodes
B = 8192
NCORES = 8
BC = B // NCORES       # 1024 batch rows per core
K = 128                # node block size
NB = M // K            # 10 blocks
KT = IN // 128         # 8 contraction tiles for the input matmul
HALF = BC // 2         # 512

# lt packing offsets: block i occupies cols [LOFF[i], LOFF[i] + M - 128*i)
LOFF = [0]
for _i in range(1, NB):
    LOFF.append(LOFF[-1] + (M - 128 * (_i - 1)))
LTOT = LOFF[-1] + (M - 128 * (NB - 1))  # 7040

# wx packing: group g0 = block 0 (ktile t at 128t), g1 = blocks 1-4
# (ktile t at 1024 + 512t), g2 = blocks 5-9 (ktile t at 5120 + 640t)
WXTOT = KT * M  # 10240

_wsplit_ctr = [0]


class _TileContextFix(TileContext):
    """This walrus build accepts only ONE embedded sem-wait per instruction;
    split extra waits onto single-wait NOPs, and same for the exit drain."""

    def _add_instruction(self, inst):
        si = getattr(inst, "sync_info", None)
        if si is not None and si.on_wait is not None and len(si.on_wait) > 1:
            waits = list(si.on_wait)
            for w in waits[:-1]:
                _wsplit_ctr[0] += 1
                nop = mybir.InstNoOp(name=f"wsplit_{_wsplit_ctr[0]}", ins=[], outs=[])
                nop.engine = inst.engine
                nop.sync_info = mybir.SyncInfo(on_wait=[w], on_update=[])
                super()._add_instruction(nop)
            si.on_wait = waits[-1:]
        super()._add_instruction(inst)

    def _drain_and_barrier(self, tick_clock, wait_clock):
        nc = self.nc
        probe = nc.sync.nop(nofuse=True, hint="exit_wait_carrier")
        wait_clock.add_sem_waits(probe.ins, ScopedClock({None: tick_clock.global_clock}))
        si = probe.ins.sync_info
        waits = list(si.on_wait) if si is not None and si.on_wait else []
        if len(waits) > 1:
            si.on_wait = waits[:1]
            for w in waits[1:]:
                n2 = nc.sync.nop(nofuse=True, hint="exit_wait_carrier")
                if n2.ins.sync_info is None:
                    n2.ins.sync_info = mybir.SyncInfo(on_wait=[w], on_update=[])
                else:
                    n2.ins.sync_info.on_wait = [w]
        nc.sync.drain()
        nc.all_engine_barrier()
        assert self.sems is not None
        popped = nc._tile_sem_poison_stack.pop()
        assert popped is self._sem_poison
        nc.clear_and_free_semaphores(list(self.sems.allocated().values()))
        nc.all_engine_barrier()


def _build():
    nc = bass.Bass("TRN2", target_bir_lowering=False, debug=False, num_devices=NCORES)

    # host-rearranged inputs, already in SBUF tile layout
    xtr = nc.dram_tensor("xtr", [128, 2 * KT * HALF], BF16, kind="ExternalInput")
    wxr = nc.dram_tensor("wxr", [128, WXTOT], BF16, kind="ExternalInput")
    ltr = nc.dram_tensor("ltr", [128, LTOT], BF16, kind="ExternalInput")
    btr = nc.dram_tensor("btr", [128, NB], F32, kind="ExternalInput")
    yT = nc.dram_tensor("yT", [OUT, BC], F32, kind="ExternalOutput")

    with _TileContextFix(nc) as tc:
        with (
            tc.tile_pool(name="sb", bufs=1) as sb,
        ):
            # persistent SBUF tiles (same layouts as the dram tensors)
            xt = sb.tile([128, 2 * KT * HALF], BF16, name="xt", tag="xt")
            wx = sb.tile([128, WXTOT], BF16, name="wx", tag="wx")
            lt = sb.tile([128, LTOT], BF16, name="lt", tag="lt")
            hb = sb.tile([128, NB * BC], BF16, name="hb", tag="hb")
            bt = sb.tile([128, NB], F32, name="bt", tag="bt")

            def wx_ap(t, u):
                if u == 0:
                    c = 128 * t
                elif u <= 4:
                    c = 1024 + 512 * t + 128 * (u - 1)
                else:
                    c = 5120 + 640 * t + 128 * (u - 5)
                return wx[:, c:c + 128]

            def xt_ap(t, h):
                c = 1024 * t + 512 * h
                return xt[:, c:c + 512]

            def ltd_ap(u):
                return lt[:, LOFF[u]:LOFF[u] + 128]

            def ltx_ap(u, i):
                c = LOFF[i] + 128 * (u - i)
                return lt[:, c:c + 128]

            # ---- DMA in: contiguous col-slice copies, big partition lines.
            # Order: what block 0 needs first; x chunks alternate between the
            # sync/gpsimd hw queues so both batch halves land early; scalar
            # only issues before its ACT stream begins.
            nc.scalar.dma_start(out=bt[:], in_=btr.ap()[:, :])
            nc.scalar.dma_start(out=wx[:, 0:1024], in_=wxr.ap()[:, 0:1024])
            for j in range(4):
                eng = (nc.sync, nc.gpsimd)[j % 2]
                eng.dma_start(out=xt[:, 2048 * j:2048 * (j + 1)],
                              in_=xtr.ap()[:, 2048 * j:2048 * (j + 1)])
            nc.scalar.dma_start(
                out=lt[:, LOFF[0]:LOFF[2]], in_=ltr.ap()[:, LOFF[0]:LOFF[2]])
            nc.sync.dma_start(out=wx[:, 1024:5120], in_=wxr.ap()[:, 1024:5120])
            nc.gpsimd.dma_start(out=wx[:, 5120:10240], in_=wxr.ap()[:, 5120:10240])
            nc.sync.dma_start(
                out=lt[:, LOFF[2]:LOFF[5]], in_=ltr.ap()[:, LOFF[2]:LOFF[5]])
            nc.gpsimd.dma_start(
                out=lt[:, LOFF[5]:LTOT], in_=ltr.ap()[:, LOFF[5]:LTOT])

            with (
                tc.tile_pool(name="pp_pool", bufs=3, space="PSUM") as pp_pool,
                tc.tile_pool(name="wu_pool", bufs=1, space="PSUM") as wu_pool,
                tc.tile_pool(name="ht0_pool", bufs=2) as ht0_pool,
                tc.tile_pool(name="ht1_pool", bufs=2) as ht1_pool,
                tc.tile_pool(name="dt_pool", bufs=2) as dt_pool,
                tc.tile_pool(name="y_pool", bufs=2) as y_pool,
            ):
                # PE warmup while the x DMA is in flight: ~4us of dummy
                # matmuls flips the HAM clock gate to 8/8 before real work,
                # so block 0's matmuls run at 2.4 GHz instead of 1.2.
                wup = sb.tile([128, 128], BF16, name="wup", tag="wup")
                nc.vector.memset(wup[:], 0.25)
                wps = wu_pool.tile([128, 128], F32, name="wps", tag="wps")
                for i in range(36):
                    nc.tensor.matmul(wps[:], wup[:], wup[:], start=True, stop=True)
                def mm_input(u, t_ps, t, h, start):
                    sl = slice(HALF * h, HALF * (h + 1))
                    nc.tensor.matmul(
                        t_ps[:, sl], wx_ap(t, u), xt_ap(t, h),
                        start=start, stop=False)

                def mm_cross(u, t_ps, i, h, last=False):
                    sl = slice(HALF * h, HALF * (h + 1))
                    nc.tensor.matmul(
                        t_ps[:, sl], ltx_ap(u, i),
                        hb[:, BC * i + HALF * h:BC * i + HALF * (h + 1)],
                        start=False, stop=last)

                def mm_iter(u, t_ps, rhs, h):
                    sl = slice(HALF * h, HALF * (h + 1))
                    nc.tensor.matmul(
                        t_ps[:, sl], ltd_ap(u), rhs[:, sl],
                        start=False, stop=False, skip_group_check=True)

                t_cur = pp_pool.tile([128, BC], F32, name="pp", tag="pp")
                for t in range(KT):
                    for h in range(2):
                        mm_input(0, t_cur, t, h, start=(t == 0))

                yhalf = []   # deferred sigmoid work: (ub, h)

                def emit_sigmoid(ub, h, ytiles):
                    if ub not in ytiles:
                        ytiles[ub] = y_pool.tile([128, BC], F32, name="y", tag="y")
                    yt = ytiles[ub]
                    sl = slice(HALF * h, HALF * (h + 1))
                    nc.scalar.activation(
                        yt[:, sl], hb[:, BC * ub + HALF * h:BC * ub + HALF * (h + 1)],
                        AF.Sigmoid)
                    return yt

                ytiles = {}
                for u in range(NB):
                    bcol = bt[:, u:u + 1]
                    last = u + 1 >= NB

                    # prefetch MM stream for block u+1, split into chunks that
                    # sandwich the iteration matmuls (keeps PE FIFO fed while
                    # the tanh chain runs, without delaying the chain)
                    pre = []
                    t_nxt = None
                    if not last:
                        t_nxt = pp_pool.tile([128, BC], F32, name="pp", tag="pp")
                        for t in range(KT):
                            for h in range(2):
                                pre.append(("in", t, h, t == 0))
                        for i in range(u):
                            for h in range(2):
                                pre.append(("x", i, h, False))

                    def emit_pre(n):
                        for _ in range(n):
                            if not pre:
                                return
                            kind, a, h, s = pre.pop(0)
                            if kind == "in":
                                mm_input(u + 1, t_nxt, a, h, start=s)
                            else:
                                mm_cross(u + 1, t_nxt, a, h)

                    # B. h0 = tanh(t + b)
                    ht0 = ht0_pool.tile([128, BC], BF16, name="ht0", tag="ht0")
                    for h in range(2):
                        sl = slice(HALF * h, HALF * (h + 1))
                        nc.scalar.activation(ht0[:, sl], t_cur[:, sl], AF.Tanh, bias=bcol)
                    if u == NB - 1:
                        # slot block NB-2's sigmoid halves into the chain
                        # bubbles (ACT idles while the iter matmuls run)
                        emit_sigmoid(NB - 2, 0, ytiles)

                    emit_pre(6)                 # ~1.3us of PE work before iter1
                    mm_iter(u, t_cur, ht0, 0)   # C
                    mm_iter(u, t_cur, ht0, 1)

                    # D. h1 = tanh(t + b)
                    ht1 = ht1_pool.tile([128, BC], BF16, name="ht1", tag="ht1")
                    for h in range(2):
                        sl = slice(HALF * h, HALF * (h + 1))
                        nc.scalar.activation(ht1[:, sl], t_cur[:, sl], AF.Tanh, bias=bcol)
                    if u == NB - 1:
                        emit_sigmoid(NB - 2, 1, ytiles)

                    # E. d = h1 - h0 (bf16 DVE)
                    dt = dt_pool.tile([128, BC], BF16, name="dt", tag="dt")
                    for h in range(2):
                        sl = slice(HALF * h, HALF * (h + 1))
                        nc.vector.tensor_tensor(
                            out=dt[:, sl], in0=ht1[:, sl], in1=ht0[:, sl], op=ALU.subtract)

                    emit_pre(5)                 # cover the h1+sub latency
                    mm_iter(u, t_cur, dt, 0)    # F
                    mm_iter(u, t_cur, dt, 1)

                    # G. h2 = tanh(t + b) -> final
                    for h in range(2):
                        sl = slice(HALF * h, HALF * (h + 1))
                        nc.scalar.activation(
                            hb[:, BC * u + HALF * h:BC * u + HALF * (h + 1)],
                            t_cur[:, sl], AF.Tanh, bias=bcol)
                        if u == NB - 1:
                            # final outputs right behind the last tanh halves
                            emit_sigmoid(NB - 1, h, ytiles)

                    emit_pre(len(pre))          # drain remaining prefetch
                    if not last:
                        # adjacent cross u+1 <- u (needs h2)
                        for h in range(2):
                            mm_cross(u + 1, t_nxt, u, h, last=(h == 1))
                        t_cur = t_nxt

                # DMA out
                y8 = ytiles[NB - 2]
                y9 = ytiles[NB - 1]
                nc.gpsimd.dma_start(out=yT.ap()[0:128, :], in_=y8[:])
                nc.gpsimd.dma_start(out=yT.ap()[128:256, 0:HALF], in_=y9[:, 0:HALF])
                nc.gpsimd.dma_start(out=yT.ap()[128:256, HALF:], in_=y9[:, HALF:])
    return nc


_nc_cache = None
BF16NP = ml_dtypes.bfloat16


def _prep(x, W, b):
    """Rearrange full inputs into per-core SBUF-layout arrays."""
    x = np.asarray(x, dtype=np.float32)
    W = np.asarray(W, dtype=np.float32)
    b = np.asarray(b, dtype=np.float32)

    # wxr: [128, 10240] — W[:, :IN].T grouped (see _build)
    WxT = W[:, :IN].T.astype(BF16NP)                  # [IN, M]
    A = WxT.reshape(KT, 128, M)                       # [t, p, m]
    g0 = A[:, :, 0:128].transpose(1, 0, 2).reshape(128, -1)
    g1 = A[:, :, 128:640].transpose(1, 0, 2).reshape(128, -1)
    g2 = A[:, :, 640:1280].transpose(1, 0, 2).reshape(128, -1)
    wxr = np.ascontiguousarray(np.concatenate([g0, g1, g2], axis=1))

    # ltr: [128, 7040] — strictly-lower L blocks, row-block i cols [128i:M)
    LT = np.triu(W[:, IN:].T, 1).astype(BF16NP)       # [M, M]
    ltr = np.ascontiguousarray(np.concatenate(
        [LT[128 * i:128 * (i + 1), 128 * i:] for i in range(NB)], axis=1))

    # btr: [128, NB]
    btr = np.ascontiguousarray(b.reshape(NB, 128).T)

    # xtr per core: [128, 8192], cols = 1024t + 512h + c
    xb = x.astype(BF16NP)
    xtrs = []
    for c in range(NCORES):
        xTc = np.ascontiguousarray(xb[c * BC:(c + 1) * BC].T)   # [IN, BC]
        arr = xTc.reshape(KT, 128, 2, HALF)                     # [t, p, h, c]
        xtrs.append(np.ascontiguousarray(
            arr.transpose(1, 0, 2, 3).reshape(128, -1)))
    return xtrs, wxr, ltr, btr


def kernel(x, W, b):
    global _nc_cache
    xtrs, wxr, ltr, btr = _prep(x, W, b)

    if _nc_cache is None:
        _nc_cache = _build()

    in_maps = [
        {"xtr": xtrs[c], "wxr": wxr, "ltr": ltr, "btr": btr}
        for c in range(NCORES)
    ]
    res = run_bass_kernel_spmd(_nc_cache, in_maps, list(range(NCORES)))
    y = np.concatenate(
        [np.ascontiguousarray(res.results[c]["yT"].T) for c in range(NCORES)], axis=0)
    return y


# revision 16
# speedup vs baseline: 1.8971x; 1.0138x over previous
"""Trainium2 Bass kernel for nn_DAG_72782515798738.

Math: node j (of M=1280) computes h_j = tanh(b_j + sum_{k<IN+j} W[j,k]*state_k)
over states = [x (IN=1024), h (M)], batch B=8192. Output y = sigmoid(h[HID:]).

Strategy: data-parallel over batch (8 cores x 1024 rows). Per core, the node
recurrence is solved block-by-block (10 blocks of 128 nodes, NODE-major tiles
[node, batch]). All matmuls run in bf16 (PSUM accumulates fp32); the 2e-2
correctness gate leaves ample room (measured ~6e-3). Per block the
pre-activation accumulator t lives in PSUM: prefetched input/cross matmuls
build t = p, then two fixed-point refinements accumulate in place:
  h0 = tanh(t+b);  t += Ld@h0;  h1 = tanh(t+b);  t += Ld@(h1-h0);  h2 = tanh(t+b)
so the only non-PE work on the chain is the tanh (ACT) and one bf16 DVE sub.

All inputs are pre-rearranged on the host into the exact SBUF tile layouts so
every DMA is a fully-contiguous col-slice copy (8-16KB per partition line).
Prefetch matmuls for block u+1 are interleaved around the iteration matmuls
of block u so the PE FIFO never stalls on the activation chain.
"""
import numpy as np
import ml_dtypes

import concourse.bass as bass
import concourse.mybir as mybir
from concourse.tile import TileContext
from concourse.vector_clock import ScopedClock
from concourse.bass_utils import run_bass_kernel_spmd

F32 = mybir.dt.float32
BF16 = mybir.dt.bfloat16
AF = mybir.ActivationFunctionType
ALU = mybir.AluOpType

IN, HID, OUT = 1024, 1024, 256
M = HID + OUT          # 1280 computed nodes
B = 8192
NCORES = 8
BC = B // NCORES       # 1024 batch rows per core
K = 128                # node block size
NB = M // K            # 10 blocks
KT = IN // 128         # 8 contraction tiles for the input matmul
HALF = BC // 2         # 512

# lt packing offsets: block i occupies cols [LOFF[i], LOFF[i] + M - 128*i)
LOFF = [0]
for _i in range(1, NB):
    LOFF.append(LOFF[-1] + (M - 128 * (_i - 1)))
LTOT = LOFF[-1] + (M - 128 * (NB - 1))  # 7040

# wx packing: group g0 = block 0 (ktile t at 128t), g1 = blocks 1-4
# (ktile t at 1024 + 512t), g2 = blocks 5-9 (ktile t at 5120 + 640t)
WXTOT = KT * M  # 10240

_wsplit_ctr = [0]


class _TileContextFix(TileContext):
    """This walrus build accepts only ONE embedded sem-wait per instruction;
    split extra waits onto single-wait NOPs, and same for the exit drain."""

    def _add_instruction(self, inst):
        si = getattr(inst, "sync_info", None)
        if si is not None and si.on_wait is not None and len(si.on_wait) > 1:
            waits = list(si.on_wait)
            for w in waits[:-1]:
                _wsplit_ctr[0] += 1
                nop = mybir.InstNoOp(name=f"wsplit_{_wsplit_ctr[0]}", ins=[], outs=[])
                nop.engine = inst.engine
                nop.sync_info = mybir.SyncInfo(on_wait=[w], on_update=[])
                super()._add_instruction(nop)
            si.on_wait = waits[-1:]
        super()._add_instruction(inst)

    def _drain_and_barrier(self, tick_clock, wait_clock):
        nc = self.nc
        probe = nc.sync.nop(nofuse=True, hint="exit_wait_carrier")
        wait_clock.add_sem_waits(probe.ins, ScopedClock({None: tick_clock.global_clock}))
        si = probe.ins.sync_info
        waits = list(si.on_wait) if si is not None and si.on_wait else []
        if len(waits) > 1:
            si.on_wait = waits[:1]
            for w in waits[1:]:
                n2 = nc.sync.nop(nofuse=True, hint="exit_wait_carrier")
                if n2.ins.sync_info is None:
                    n2.ins.sync_info = mybir.SyncInfo(on_wait=[w], on_update=[])
                else:
                    n2.ins.sync_info.on_wait = [w]
        nc.sync.drain()
        nc.all_engine_barrier()
        assert self.sems is not None
        popped = nc._tile_sem_poison_stack.pop()
        assert popped is self._sem_poison
        nc.clear_and_free_semaphores(list(self.sems.allocated().values()))
        nc.all_engine_barrier()


def _build():
    nc = bass.Bass("TRN2", target_bir_lowering=False, debug=False, num_devices=NCORES)

    # host-rearranged inputs, already in SBUF tile layout
    xtr = nc.dram_tensor("xtr", [128, 2 * KT * HALF], BF16, kind="ExternalInput")
    wxr = nc.dram_tensor("wxr", [128, WXTOT], BF16, kind="ExternalInput")
    ltr = nc.dram_tensor("ltr", [128, LTOT], BF16, kind="ExternalInput")
    btr = nc.dram_tensor("btr", [128, NB], F32, kind="ExternalInput")
    yT = nc.dram_tensor("yT", [OUT, BC], F32, kind="ExternalOutput")

    with _TileContextFix(nc) as tc:
        with (
            tc.tile_pool(name="sb", bufs=1) as sb,
        ):
            # persistent SBUF tiles. One tile per independently-DMA'd chunk:
            # Tile makes any reader wait on ALL writers of a tile, so a
            # consumer must share a tile only with the DMA that feeds it.
            xtc = [sb.tile([128, 2048], BF16, name=f"xt{j}", tag=f"xt{j}")
                   for j in range(4)]                     # ktile pair j
            wxu = [sb.tile([128, KT * 128], BF16, name=f"wx{u}", tag=f"wx{u}")
                   for u in range(NB)]                    # input weights, block u
            ltu = [sb.tile([128, M - 128 * i], BF16, name=f"lt{i}", tag=f"lt{i}")
                   for i in range(NB)]                    # L rows of block i
            hb = sb.tile([128, NB * BC], BF16, name="hb", tag="hb")
            bt = sb.tile([128, NB], F32, name="bt", tag="bt")

            def wx_ap(t, u):
                return wxu[u][:, 128 * t:128 * (t + 1)]

            def xt_ap(t, h):
                c = 1024 * (t % 2) + 512 * h
                return xtc[t // 2][:, c:c + 512]

            def ltd_ap(u):
                return ltu[u][:, 0:128]

            def ltx_ap(u, i):
                return ltu[i][:, 128 * (u - i):128 * (u - i + 1)]

            # ---- DMA in: contiguous col-slice copies, big partition lines,
            # ordered just-in-time across the three DGE queues (sync=SP,
            # gpsimd=SWDGE, scalar=Activation HWDGE). x first (gates block
            # 0), then wx/lt interleaved by first-use time. scalar's issue
            # count is capped so its ACT stream starts on time.
            nc.scalar.dma_start(out=bt[:], in_=btr.ap()[:, :])
            nc.scalar.dma_start(out=wxu[0][:], in_=wxr.ap()[:, 0:1024])
            nc.sync.dma_start(out=xtc[0][:], in_=xtr.ap()[:, 0:2048])
            nc.gpsimd.dma_start(out=xtc[1][:], in_=xtr.ap()[:, 2048:4096])
            nc.scalar.dma_start(out=xtc[2][:], in_=xtr.ap()[:, 4096:6144])
            nc.sync.dma_start(out=xtc[3][:], in_=xtr.ap()[:, 6144:8192])

            def dma_wx(eng, u):
                eng.dma_start(out=wxu[u][:],
                              in_=wxr.ap()[:, 1024 * u:1024 * (u + 1)])

            def dma_lt(eng, i):
                eng.dma_start(out=ltu[i][:],
                              in_=ltr.ap()[:, LOFF[i]:LOFF[i] + M - 128 * i])

            dma_wx(nc.gpsimd, 1)
            dma_lt(nc.sync, 0)
            dma_wx(nc.gpsimd, 2)
            dma_lt(nc.sync, 1)
            dma_wx(nc.gpsimd, 3)
            dma_lt(nc.sync, 2)
            dma_wx(nc.gpsimd, 4)
            dma_wx(nc.scalar, 5)
            dma_lt(nc.sync, 3)
            dma_wx(nc.gpsimd, 6)
            dma_lt(nc.sync, 4)
            dma_wx(nc.gpsimd, 7)
            dma_lt(nc.sync, 5)
            dma_wx(nc.gpsimd, 8)
            dma_lt(nc.scalar, 6)
            dma_wx(nc.sync, 9)
            dma_lt(nc.gpsimd, 7)
            dma_lt(nc.sync, 8)
            dma_lt(nc.gpsimd, 9)

            with (
                tc.tile_pool(name="pp_pool", bufs=3, space="PSUM") as pp_pool,
                tc.tile_pool(name="wu_pool", bufs=1, space="PSUM") as wu_pool,
                tc.tile_pool(name="ht0_pool", bufs=2) as ht0_pool,
                tc.tile_pool(name="ht1_pool", bufs=2) as ht1_pool,
                tc.tile_pool(name="dt_pool", bufs=2) as dt_pool,
                tc.tile_pool(name="y_pool", bufs=2) as y_pool,
            ):
                # PE warmup while the x DMA is in flight: ~4us of dummy
                # matmuls flips the HAM clock gate to 8/8 before real work,
                # so block 0's matmuls run at 2.4 GHz instead of 1.2.
                wup = sb.tile([128, 128], BF16, name="wup", tag="wup")
                nc.vector.memset(wup[:], 0.25)
                wps = wu_pool.tile([128, 128], F32, name="wps", tag="wps")
                for i in range(44):
                    nc.tensor.matmul(wps[:], wup[:], wup[:], start=True, stop=True)
                def mm_input(u, t_ps, t, h, start):
                    sl = slice(HALF * h, HALF * (h + 1))
                    nc.tensor.matmul(
                        t_ps[:, sl], wx_ap(t, u), xt_ap(t, h),
                        start=start, stop=False)

                def mm_cross(u, t_ps, i, h, last=False):
                    sl = slice(HALF * h, HALF * (h + 1))
                    nc.tensor.matmul(
                        t_ps[:, sl], ltx_ap(u, i),
                        hb[:, BC * i + HALF * h:BC * i + HALF * (h + 1)],
                        start=False, stop=last)

                def mm_iter(u, t_ps, rhs, h):
                    sl = slice(HALF * h, HALF * (h + 1))
                    nc.tensor.matmul(
                        t_ps[:, sl], ltd_ap(u), rhs[:, sl],
                        start=False, stop=False, skip_group_check=True)

                t_cur = pp_pool.tile([128, BC], F32, name="pp", tag="pp")
                for t in range(KT):
                    for h in range(2):
                        mm_input(0, t_cur, t, h, start=(t == 0))

                yhalf = []   # deferred sigmoid work: (ub, h)

                def emit_sigmoid(ub, h, ytiles):
                    if ub not in ytiles:
                        ytiles[ub] = y_pool.tile([128, BC], F32, name="y", tag="y")
                    yt = ytiles[ub]
                    sl = slice(HALF * h, HALF * (h + 1))
                    nc.scalar.activation(
                        yt[:, sl], hb[:, BC * ub + HALF * h:BC * ub + HALF * (h + 1)],
                        AF.Sigmoid)
                    return yt

                ytiles = {}
                for u in range(NB):
                    bcol = bt[:, u:u + 1]
                    last = u + 1 >= NB

                    # prefetch MM stream for block u+1, split into chunks that
                    # sandwich the iteration matmuls (keeps PE FIFO fed while
                    # the tanh chain runs, without delaying the chain)
                    pre = []
                    t_nxt = None
                    if not last:
                        t_nxt = pp_pool.tile([128, BC], F32, name="pp", tag="pp")
                        for t in range(KT):
                            for h in range(2):
                                pre.append(("in", t, h, t == 0))
                        for i in range(u):
                            for h in range(2):
                                pre.append(("x", i, h, False))

                    def emit_pre(n):
                        for _ in range(n):
                            if not pre:
                                return
                            kind, a, h, s = pre.pop(0)
                            if kind == "in":
                                mm_input(u + 1, t_nxt, a, h, start=s)
                            else:
                                mm_cross(u + 1, t_nxt, a, h)

                    # B. h0 = tanh(t + b)
                    ht0 = ht0_pool.tile([128, BC], BF16, name="ht0", tag="ht0")
                    for h in range(2):
                        sl = slice(HALF * h, HALF * (h + 1))
                        nc.scalar.activation(ht0[:, sl], t_cur[:, sl], AF.Tanh, bias=bcol)
                    if u == NB - 1:
                        # slot block NB-2's sigmoid halves into the chain
                        # bubbles (ACT idles while the iter matmuls run)
                        emit_sigmoid(NB - 2, 0, ytiles)

                    emit_pre(6)                 # ~1.3us of PE work before iter1
                    mm_iter(u, t_cur, ht0, 0)   # C
                    mm_iter(u, t_cur, ht0, 1)

                    # D. h1 = tanh(t + b)
                    ht1 = ht1_pool.tile([128, BC], BF16, name="ht1", tag="ht1")
                    for h in range(2):
                        sl = slice(HALF * h, HALF * (h + 1))
                        nc.scalar.activation(ht1[:, sl], t_cur[:, sl], AF.Tanh, bias=bcol)
                    if u == NB - 1:
                        emit_sigmoid(NB - 2, 1, ytiles)

                    # E. d = h1 - h0 (bf16 DVE)
                    dt = dt_pool.tile([128, BC], BF16, name="dt", tag="dt")
                    for h in range(2):
                        sl = slice(HALF * h, HALF * (h + 1))
                        nc.vector.tensor_tensor(
                            out=dt[:, sl], in0=ht1[:, sl], in1=ht0[:, sl], op=ALU.subtract)

                    emit_pre(5)                 # cover the h1+sub latency
                    mm_iter(u, t_cur, dt, 0)    # F
                    mm_iter(u, t_cur, dt, 1)

                    # G. h2 = tanh(t + b) -> final
                    for h in range(2):
                        sl = slice(HALF * h, HALF * (h + 1))
                        nc.scalar.activation(
                            hb[:, BC * u + HALF * h:BC * u + HALF * (h + 1)],
                            t_cur[:, sl], AF.Tanh, bias=bcol)
                        if u == NB - 1:
                            # final outputs right behind the last tanh halves
                            emit_sigmoid(NB - 1, h, ytiles)

                    emit_pre(len(pre))          # drain remaining prefetch
                    if not last:
                        # adjacent cross u+1 <- u (needs h2)
                        for h in range(2):
                            mm_cross(u + 1, t_nxt, u, h, last=(h == 1))
                        t_cur = t_nxt

                # DMA out
                y8 = ytiles[NB - 2]
                y9 = ytiles[NB - 1]
                nc.gpsimd.dma_start(out=yT.ap()[0:128, :], in_=y8[:])
                nc.gpsimd.dma_start(out=yT.ap()[128:256, 0:HALF], in_=y9[:, 0:HALF])
                nc.gpsimd.dma_start(out=yT.ap()[128:256, HALF:], in_=y9[:, HALF:])
    return nc


_nc_cache = None
BF16NP = ml_dtypes.bfloat16


def _prep(x, W, b):
    """Rearrange full inputs into per-core SBUF-layout arrays."""
    x = np.asarray(x, dtype=np.float32)
    W = np.asarray(W, dtype=np.float32)
    b = np.asarray(b, dtype=np.float32)

    # wxr: [128, 10240] — W[:, :IN].T block-major: cols = 1024u + 128t + c
    WxT = W[:, :IN].T.astype(BF16NP)                  # [IN, M]
    A = WxT.reshape(KT, 128, M)                       # [t, p, m]
    wxr = np.ascontiguousarray(np.concatenate(
        [A[:, :, 128 * u:128 * (u + 1)].transpose(1, 0, 2).reshape(128, -1)
         for u in range(NB)], axis=1))

    # ltr: [128, 7040] — strictly-lower L blocks, row-block i cols [128i:M)
    LT = np.triu(W[:, IN:].T, 1).astype(BF16NP)       # [M, M]
    ltr = np.ascontiguousarray(np.concatenate(
        [LT[128 * i:128 * (i + 1), 128 * i:] for i in range(NB)], axis=1))

    # btr: [128, NB]
    btr = np.ascontiguousarray(b.reshape(NB, 128).T)

    # xtr per core: [128, 8192], cols = 1024t + 512h + c
    xb = x.astype(BF16NP)
    xtrs = []
    for c in range(NCORES):
        xTc = np.ascontiguousarray(xb[c * BC:(c + 1) * BC].T)   # [IN, BC]
        arr = xTc.reshape(KT, 128, 2, HALF)                     # [t, p, h, c]
        xtrs.append(np.ascontiguousarray(
            arr.transpose(1, 0, 2, 3).reshape(128, -1)))
    return xtrs, wxr, ltr, btr


def kernel(x, W, b):
    global _nc_cache
    xtrs, wxr, ltr, btr = _prep(x, W, b)

    if _nc_cache is None:
        _nc_cache = _build()

    in_maps = [
        {"xtr": xtrs[c], "wxr": wxr, "ltr": ltr, "btr": btr}
        for c in range(NCORES)
    ]
    res = run_bass_kernel_spmd(_nc_cache, in_maps, list(range(NCORES)))
    y = np.concatenate(
        [np.ascontiguousarray(res.results[c]["yT"].T) for c in range(NCORES)], axis=0)
    return y


# revision 17
# speedup vs baseline: 1.9204x; 1.0123x over previous
"""Trainium2 Bass kernel for nn_DAG_72782515798738.

Math: node j (of M=1280) computes h_j = tanh(b_j + sum_{k<IN+j} W[j,k]*state_k)
over states = [x (IN=1024), h (M)], batch B=8192. Output y = sigmoid(h[HID:]).

Strategy: data-parallel over batch (8 cores x 1024 rows). Per core, the node
recurrence is solved block-by-block (10 blocks of 128 nodes, NODE-major tiles
[node, batch]). All matmuls run in bf16 (PSUM accumulates fp32); the 2e-2
correctness gate leaves ample room (measured ~6e-3). Per block the
pre-activation accumulator t lives in PSUM: prefetched input/cross matmuls
build t = p, then two fixed-point refinements accumulate in place:
  h0 = tanh(t+b);  t += Ld@h0;  h1 = tanh(t+b);  t += Ld@(h1-h0);  h2 = tanh(t+b)
so the only non-PE work on the chain is the tanh (ACT) and one bf16 DVE sub.

All inputs are pre-rearranged on the host into the exact SBUF tile layouts so
every DMA is a fully-contiguous col-slice copy (8-16KB per partition line).
Prefetch matmuls for block u+1 are interleaved around the iteration matmuls
of block u so the PE FIFO never stalls on the activation chain.
"""
import numpy as np
import ml_dtypes

import concourse.bass as bass
import concourse.mybir as mybir
from concourse.tile import TileContext
from concourse.vector_clock import ScopedClock
from concourse.bass_utils import run_bass_kernel_spmd

F32 = mybir.dt.float32
BF16 = mybir.dt.bfloat16
AF = mybir.ActivationFunctionType
ALU = mybir.AluOpType

IN, HID, OUT = 1024, 1024, 256
M = HID + OUT          # 1280 computed nodes
B = 8192
NCORES = 8
BC = B // NCORES       # 1024 batch rows per core
K = 128                # node block size
NB = M // K            # 10 blocks
KT = IN // 128         # 8 contraction tiles for the input matmul
HALF = BC // 2         # 512

# lt packing offsets: block i occupies cols [LOFF[i], LOFF[i] + M - 128*i)
LOFF = [0]
for _i in range(1, NB):
    LOFF.append(LOFF[-1] + (M - 128 * (_i - 1)))
LTOT = LOFF[-1] + (M - 128 * (NB - 1))  # 7040

# wx packing: group g0 = block 0 (ktile t at 128t), g1 = blocks 1-4
# (ktile t at 1024 + 512t), g2 = blocks 5-9 (ktile t at 5120 + 640t)
WXTOT = KT * M  # 10240

_wsplit_ctr = [0]


class _TileContextFix(TileContext):
    """This walrus build accepts only ONE embedded sem-wait per instruction;
    split extra waits onto single-wait NOPs, and same for the exit drain."""

    def _add_instruction(self, inst):
        si = getattr(inst, "sync_info", None)
        if si is not None and si.on_wait is not None and len(si.on_wait) > 1:
            waits = list(si.on_wait)
            for w in waits[:-1]:
                _wsplit_ctr[0] += 1
                nop = mybir.InstNoOp(name=f"wsplit_{_wsplit_ctr[0]}", ins=[], outs=[])
                nop.engine = inst.engine
                nop.sync_info = mybir.SyncInfo(on_wait=[w], on_update=[])
                super()._add_instruction(nop)
            si.on_wait = waits[-1:]
        super()._add_instruction(inst)

    def _drain_and_barrier(self, tick_clock, wait_clock):
        nc = self.nc
        probe = nc.sync.nop(nofuse=True, hint="exit_wait_carrier")
        wait_clock.add_sem_waits(probe.ins, ScopedClock({None: tick_clock.global_clock}))
        si = probe.ins.sync_info
        waits = list(si.on_wait) if si is not None and si.on_wait else []
        if len(waits) > 1:
            si.on_wait = waits[:1]
            for w in waits[1:]:
                n2 = nc.sync.nop(nofuse=True, hint="exit_wait_carrier")
                if n2.ins.sync_info is None:
                    n2.ins.sync_info = mybir.SyncInfo(on_wait=[w], on_update=[])
                else:
                    n2.ins.sync_info.on_wait = [w]
        nc.sync.drain()
        nc.all_engine_barrier()
        assert self.sems is not None
        popped = nc._tile_sem_poison_stack.pop()
        assert popped is self._sem_poison
        nc.clear_and_free_semaphores(list(self.sems.allocated().values()))
        nc.all_engine_barrier()


def _build():
    nc = bass.Bass("TRN2", target_bir_lowering=False, debug=False, num_devices=NCORES)

    # host-rearranged inputs, already in SBUF tile layout
    xtr = nc.dram_tensor("xtr", [128, 2 * KT * HALF], BF16, kind="ExternalInput")
    wxr = nc.dram_tensor("wxr", [128, WXTOT], BF16, kind="ExternalInput")
    ltr = nc.dram_tensor("ltr", [128, LTOT], BF16, kind="ExternalInput")
    btr = nc.dram_tensor("btr", [128, NB], F32, kind="ExternalInput")
    yT = nc.dram_tensor("yT", [OUT, BC], F32, kind="ExternalOutput")

    with _TileContextFix(nc) as tc:
        with (
            tc.tile_pool(name="sb", bufs=1) as sb,
        ):
            # persistent SBUF tiles. One tile per independently-DMA'd chunk:
            # Tile makes any reader wait on ALL writers of a tile, so a
            # consumer must share a tile only with the DMA that feeds it.
            xtc = [sb.tile([128, 2048], BF16, name=f"xt{j}", tag=f"xt{j}")
                   for j in range(4)]                     # ktile pair j
            wxu = [sb.tile([128, KT * 128], BF16, name=f"wx{u}", tag=f"wx{u}")
                   for u in range(NB)]                    # input weights, block u
            ltu = [sb.tile([128, M - 128 * i], BF16, name=f"lt{i}", tag=f"lt{i}")
                   for i in range(NB)]                    # L rows of block i
            hb = sb.tile([128, NB * BC], BF16, name="hb", tag="hb")
            bt = sb.tile([128, NB], F32, name="bt", tag="bt")

            def wx_ap(t, u):
                return wxu[u][:, 128 * t:128 * (t + 1)]

            def xt_ap(t, h):
                c = 1024 * (t % 2) + 512 * h
                return xtc[t // 2][:, c:c + 512]

            def ltd_ap(u):
                return ltu[u][:, 0:128]

            def ltx_ap(u, i):
                return ltu[i][:, 128 * (u - i):128 * (u - i + 1)]

            # ---- DMA in: contiguous col-slice copies, big partition lines.
            # The DMA fabric serves ALL queued transfers concurrently
            # (~320 GB/s shared), so priority needs gating: only x (which
            # gates block 0) + tiny bt/wx0 are enqueued at first. The bulk
            # wx/lt stream sits behind tiny gpsimd copies that READ the x
            # tiles, so it only starts once x has fully landed.
            nc.scalar.dma_start(out=bt[:], in_=btr.ap()[:, :])
            nc.scalar.dma_start(out=wxu[0][:], in_=wxr.ap()[:, 0:1024])
            nc.sync.dma_start(out=xtc[0][:], in_=xtr.ap()[:, 0:2048])
            nc.gpsimd.dma_start(out=xtc[1][:], in_=xtr.ap()[:, 2048:4096])
            nc.scalar.dma_start(out=xtc[2][:], in_=xtr.ap()[:, 4096:6144])
            nc.sync.dma_start(out=xtc[3][:], in_=xtr.ap()[:, 6144:8192])

            scr = sb.tile([128, 8], BF16, name="scr", tag="scr")
            for j in range(4):
                nc.gpsimd.tensor_copy(out=scr[0:1, 2 * j:2 * j + 2],
                                      in_=xtc[j][0:1, 0:2])

            def dma_wx(eng, u):
                eng.dma_start(out=wxu[u][:],
                              in_=wxr.ap()[:, 1024 * u:1024 * (u + 1)])

            def dma_lt(eng, i):
                eng.dma_start(out=ltu[i][:],
                              in_=ltr.ap()[:, LOFF[i]:LOFF[i] + M - 128 * i])

            dma_wx(nc.gpsimd, 1)
            dma_lt(nc.gpsimd, 0)
            dma_wx(nc.gpsimd, 2)
            dma_lt(nc.gpsimd, 1)
            dma_wx(nc.gpsimd, 3)
            dma_lt(nc.gpsimd, 2)
            dma_wx(nc.gpsimd, 4)
            dma_lt(nc.gpsimd, 3)
            dma_wx(nc.gpsimd, 5)
            dma_lt(nc.gpsimd, 4)
            dma_wx(nc.gpsimd, 6)
            dma_lt(nc.gpsimd, 5)
            dma_wx(nc.gpsimd, 7)
            dma_lt(nc.gpsimd, 6)
            dma_wx(nc.gpsimd, 8)
            dma_lt(nc.gpsimd, 7)
            dma_wx(nc.gpsimd, 9)
            dma_lt(nc.gpsimd, 8)
            dma_lt(nc.gpsimd, 9)

            with (
                tc.tile_pool(name="pp_pool", bufs=3, space="PSUM") as pp_pool,
                tc.tile_pool(name="wu_pool", bufs=1, space="PSUM") as wu_pool,
                tc.tile_pool(name="ht0_pool", bufs=2) as ht0_pool,
                tc.tile_pool(name="ht1_pool", bufs=2) as ht1_pool,
                tc.tile_pool(name="dt_pool", bufs=2) as dt_pool,
                tc.tile_pool(name="y_pool", bufs=2) as y_pool,
            ):
                # PE warmup while the x DMA is in flight: ~4us of dummy
                # matmuls flips the HAM clock gate to 8/8 before real work,
                # so block 0's matmuls run at 2.4 GHz instead of 1.2.
                wup = sb.tile([128, 128], BF16, name="wup", tag="wup")
                nc.vector.memset(wup[:], 0.25)
                wps = wu_pool.tile([128, 128], F32, name="wps", tag="wps")
                for i in range(44):
                    nc.tensor.matmul(wps[:], wup[:], wup[:], start=True, stop=True)
                def mm_input(u, t_ps, t, h, start):
                    sl = slice(HALF * h, HALF * (h + 1))
                    nc.tensor.matmul(
                        t_ps[:, sl], wx_ap(t, u), xt_ap(t, h),
                        start=start, stop=False)

                def mm_cross(u, t_ps, i, h, last=False):
                    sl = slice(HALF * h, HALF * (h + 1))
                    nc.tensor.matmul(
                        t_ps[:, sl], ltx_ap(u, i),
                        hb[:, BC * i + HALF * h:BC * i + HALF * (h + 1)],
                        start=False, stop=last)

                def mm_iter(u, t_ps, rhs, h):
                    sl = slice(HALF * h, HALF * (h + 1))
                    nc.tensor.matmul(
                        t_ps[:, sl], ltd_ap(u), rhs[:, sl],
                        start=False, stop=False, skip_group_check=True)

                t_cur = pp_pool.tile([128, BC], F32, name="pp", tag="pp")
                for t in range(KT):
                    for h in range(2):
                        mm_input(0, t_cur, t, h, start=(t == 0))

                yhalf = []   # deferred sigmoid work: (ub, h)

                def emit_sigmoid(ub, h, ytiles):
                    if ub not in ytiles:
                        ytiles[ub] = y_pool.tile([128, BC], F32, name="y", tag="y")
                    yt = ytiles[ub]
                    sl = slice(HALF * h, HALF * (h + 1))
                    nc.scalar.activation(
                        yt[:, sl], hb[:, BC * ub + HALF * h:BC * ub + HALF * (h + 1)],
                        AF.Sigmoid)
                    return yt

                ytiles = {}
                for u in range(NB):
                    bcol = bt[:, u:u + 1]
                    last = u + 1 >= NB

                    # prefetch MM stream for block u+1, split into chunks that
                    # sandwich the iteration matmuls (keeps PE FIFO fed while
                    # the tanh chain runs, without delaying the chain)
                    pre = []
                    t_nxt = None
                    if not last:
                        t_nxt = pp_pool.tile([128, BC], F32, name="pp", tag="pp")
                        for t in range(KT):
                            for h in range(2):
                                pre.append(("in", t, h, t == 0))
                        for i in range(u):
                            for h in range(2):
                                pre.append(("x", i, h, False))

                    def emit_pre(n):
                        for _ in range(n):
                            if not pre:
                                return
                            kind, a, h, s = pre.pop(0)
                            if kind == "in":
                                mm_input(u + 1, t_nxt, a, h, start=s)
                            else:
                                mm_cross(u + 1, t_nxt, a, h)

                    # B. h0 = tanh(t + b)
                    ht0 = ht0_pool.tile([128, BC], BF16, name="ht0", tag="ht0")
                    for h in range(2):
                        sl = slice(HALF * h, HALF * (h + 1))
                        nc.scalar.activation(ht0[:, sl], t_cur[:, sl], AF.Tanh, bias=bcol)
                    if u == NB - 1:
                        # slot block NB-2's sigmoid halves into the chain
                        # bubbles (ACT idles while the iter matmuls run)
                        emit_sigmoid(NB - 2, 0, ytiles)

                    emit_pre(6)                 # ~1.3us of PE work before iter1
                    mm_iter(u, t_cur, ht0, 0)   # C
                    mm_iter(u, t_cur, ht0, 1)

                    # D. h1 = tanh(t + b)
                    ht1 = ht1_pool.tile([128, BC], BF16, name="ht1", tag="ht1")
                    for h in range(2):
                        sl = slice(HALF * h, HALF * (h + 1))
                        nc.scalar.activation(ht1[:, sl], t_cur[:, sl], AF.Tanh, bias=bcol)
                    if u == NB - 1:
                        emit_sigmoid(NB - 2, 1, ytiles)

                    # E. d = h1 - h0 (bf16 DVE)
                    dt = dt_pool.tile([128, BC], BF16, name="dt", tag="dt")
                    for h in range(2):
                        sl = slice(HALF * h, HALF * (h + 1))
                        nc.vector.tensor_tensor(
                            out=dt[:, sl], in0=ht1[:, sl], in1=ht0[:, sl], op=ALU.subtract)

                    emit_pre(5)                 # cover the h1+sub latency
                    mm_iter(u, t_cur, dt, 0)    # F
                    mm_iter(u, t_cur, dt, 1)

                    # G. h2 = tanh(t + b) -> final
                    for h in range(2):
                        sl = slice(HALF * h, HALF * (h + 1))
                        nc.scalar.activation(
                            hb[:, BC * u + HALF * h:BC * u + HALF * (h + 1)],
                            t_cur[:, sl], AF.Tanh, bias=bcol)
                        if u == NB - 1:
                            # final outputs right behind the last tanh halves
                            emit_sigmoid(NB - 1, h, ytiles)

                    emit_pre(len(pre))          # drain remaining prefetch
                    if not last:
                        # adjacent cross u+1 <- u (needs h2)
                        for h in range(2):
                            mm_cross(u + 1, t_nxt, u, h, last=(h == 1))
                        t_cur = t_nxt

                # DMA out
                y8 = ytiles[NB - 2]
                y9 = ytiles[NB - 1]
                nc.gpsimd.dma_start(out=yT.ap()[0:128, :], in_=y8[:])
                nc.gpsimd.dma_start(out=yT.ap()[128:256, 0:HALF], in_=y9[:, 0:HALF])
                nc.gpsimd.dma_start(out=yT.ap()[128:256, HALF:], in_=y9[:, HALF:])
    return nc


_nc_cache = None
BF16NP = ml_dtypes.bfloat16


def _prep(x, W, b):
    """Rearrange full inputs into per-core SBUF-layout arrays."""
    x = np.asarray(x, dtype=np.float32)
    W = np.asarray(W, dtype=np.float32)
    b = np.asarray(b, dtype=np.float32)

    # wxr: [128, 10240] — W[:, :IN].T block-major: cols = 1024u + 128t + c
    WxT = W[:, :IN].T.astype(BF16NP)                  # [IN, M]
    A = WxT.reshape(KT, 128, M)                       # [t, p, m]
    wxr = np.ascontiguousarray(np.concatenate(
        [A[:, :, 128 * u:128 * (u + 1)].transpose(1, 0, 2).reshape(128, -1)
         for u in range(NB)], axis=1))

    # ltr: [128, 7040] — strictly-lower L blocks, row-block i cols [128i:M)
    LT = np.triu(W[:, IN:].T, 1).astype(BF16NP)       # [M, M]
    ltr = np.ascontiguousarray(np.concatenate(
        [LT[128 * i:128 * (i + 1), 128 * i:] for i in range(NB)], axis=1))

    # btr: [128, NB]
    btr = np.ascontiguousarray(b.reshape(NB, 128).T)

    # xtr per core: [128, 8192], cols = 1024t + 512h + c
    xb = x.astype(BF16NP)
    xtrs = []
    for c in range(NCORES):
        xTc = np.ascontiguousarray(xb[c * BC:(c + 1) * BC].T)   # [IN, BC]
        arr = xTc.reshape(KT, 128, 2, HALF)                     # [t, p, h, c]
        xtrs.append(np.ascontiguousarray(
            arr.transpose(1, 0, 2, 3).reshape(128, -1)))
    return xtrs, wxr, ltr, btr


def kernel(x, W, b):
    global _nc_cache
    xtrs, wxr, ltr, btr = _prep(x, W, b)

    if _nc_cache is None:
        _nc_cache = _build()

    in_maps = [
        {"xtr": xtrs[c], "wxr": wxr, "ltr": ltr, "btr": btr}
        for c in range(NCORES)
    ]
    res = run_bass_kernel_spmd(_nc_cache, in_maps, list(range(NCORES)))
    y = np.concatenate(
        [np.ascontiguousarray(res.results[c]["yT"].T) for c in range(NCORES)], axis=0)
    return y


# revision 19
# speedup vs baseline: 1.9781x; 1.0300x over previous
"""Trainium2 Bass kernel for nn_DAG_72782515798738.

Math: node j (of M=1280) computes h_j = tanh(b_j + sum_{k<IN+j} W[j,k]*state_k)
over states = [x (IN=1024), h (M)], batch B=8192. Output y = sigmoid(h[HID:]).

Strategy: data-parallel over batch (8 cores x 1024 rows). Per core, the node
recurrence is solved block-by-block (10 blocks of 128 nodes, NODE-major tiles
[node, batch]). All matmuls run in bf16 (PSUM accumulates fp32); the 2e-2
correctness gate leaves ample room (measured ~6e-3). Per block the
pre-activation accumulator t lives in PSUM: prefetched input/cross matmuls
build t = p, then two fixed-point refinements accumulate in place:
  h0 = tanh(t+b);  t += Ld@h0;  h1 = tanh(t+b);  t += Ld@(h1-h0);  h2 = tanh(t+b)
so the only non-PE work on the chain is the tanh (ACT) and one bf16 DVE sub.

All inputs are pre-rearranged on the host into the exact SBUF tile layouts so
every DMA is a fully-contiguous col-slice copy (8-16KB per partition line).
Prefetch matmuls for block u+1 are interleaved around the iteration matmuls
of block u so the PE FIFO never stalls on the activation chain.
"""
import numpy as np
import ml_dtypes

import concourse.bass as bass
import concourse.mybir as mybir
from concourse.tile import TileContext
from concourse.vector_clock import ScopedClock
from concourse.bass_utils import run_bass_kernel_spmd

F32 = mybir.dt.float32
BF16 = mybir.dt.bfloat16
AF = mybir.ActivationFunctionType
ALU = mybir.AluOpType

IN, HID, OUT = 1024, 1024, 256
M = HID + OUT          # 1280 computed nodes
B = 8192
NCORES = 8
BC = B // NCORES       # 1024 batch rows per core
K = 128                # node block size
NB = M // K            # 10 blocks
KT = IN // 128         # 8 contraction tiles for the input matmul
HALF = BC // 2         # 512

# lt packing offsets: block i occupies cols [LOFF[i], LOFF[i] + M - 128*i)
LOFF = [0]
for _i in range(1, NB):
    LOFF.append(LOFF[-1] + (M - 128 * (_i - 1)))
LTOT = LOFF[-1] + (M - 128 * (NB - 1))  # 7040

# wx packing: group g0 = block 0 (ktile t at 128t), g1 = blocks 1-4
# (ktile t at 1024 + 512t), g2 = blocks 5-9 (ktile t at 5120 + 640t)
WXTOT = KT * M  # 10240

_wsplit_ctr = [0]


class _TileContextFix(TileContext):
    """This walrus build accepts only ONE embedded sem-wait per instruction;
    split extra waits onto single-wait NOPs, and same for the exit drain."""

    def _add_instruction(self, inst):
        si = getattr(inst, "sync_info", None)
        if si is not None and si.on_wait is not None and len(si.on_wait) > 1:
            waits = list(si.on_wait)
            for w in waits[:-1]:
                _wsplit_ctr[0] += 1
                nop = mybir.InstNoOp(name=f"wsplit_{_wsplit_ctr[0]}", ins=[], outs=[])
                nop.engine = inst.engine
                nop.sync_info = mybir.SyncInfo(on_wait=[w], on_update=[])
                super()._add_instruction(nop)
            si.on_wait = waits[-1:]
        super()._add_instruction(inst)

    def _drain_and_barrier(self, tick_clock, wait_clock):
        nc = self.nc
        probe = nc.sync.nop(nofuse=True, hint="exit_wait_carrier")
        wait_clock.add_sem_waits(probe.ins, ScopedClock({None: tick_clock.global_clock}))
        si = probe.ins.sync_info
        waits = list(si.on_wait) if si is not None and si.on_wait else []
        if len(waits) > 1:
            si.on_wait = waits[:1]
            for w in waits[1:]:
                n2 = nc.sync.nop(nofuse=True, hint="exit_wait_carrier")
                if n2.ins.sync_info is None:
                    n2.ins.sync_info = mybir.SyncInfo(on_wait=[w], on_update=[])
                else:
                    n2.ins.sync_info.on_wait = [w]
        nc.sync.drain()
        nc.all_engine_barrier()
        assert self.sems is not None
        popped = nc._tile_sem_poison_stack.pop()
        assert popped is self._sem_poison
        nc.clear_and_free_semaphores(list(self.sems.allocated().values()))
        nc.all_engine_barrier()


def _build():
    nc = bass.Bass("TRN2", target_bir_lowering=False, debug=False, num_devices=NCORES)

    # host-rearranged inputs, already in SBUF tile layout
    xtr = nc.dram_tensor("xtr", [128, 2 * KT * HALF], BF16, kind="ExternalInput")
    wxr = nc.dram_tensor("wxr", [128, WXTOT], BF16, kind="ExternalInput")
    ltr = nc.dram_tensor("ltr", [128, LTOT], BF16, kind="ExternalInput")
    btr = nc.dram_tensor("btr", [128, NB], F32, kind="ExternalInput")
    yT = nc.dram_tensor("yT", [OUT, BC], F32, kind="ExternalOutput")

    with _TileContextFix(nc) as tc:
        with (
            tc.tile_pool(name="sb", bufs=1) as sb,
        ):
            # persistent SBUF tiles. One tile per independently-DMA'd chunk:
            # Tile makes any reader wait on ALL writers of a tile, so a
            # consumer must share a tile only with the DMA that feeds it.
            xtc = [sb.tile([128, 2048], BF16, name=f"xt{j}", tag=f"xt{j}")
                   for j in range(4)]                     # ktile pair j
            wxu = [sb.tile([128, KT * 128], BF16, name=f"wx{u}", tag=f"wx{u}")
                   for u in range(NB)]                    # input weights, block u
            ltu = [sb.tile([128, M - 128 * i], BF16, name=f"lt{i}", tag=f"lt{i}")
                   for i in range(NB)]                    # L rows of block i
            hb = sb.tile([128, NB * BC], BF16, name="hb", tag="hb")
            bt = sb.tile([128, NB], F32, name="bt", tag="bt")

            def wx_ap(t, u):
                return wxu[u][:, 128 * t:128 * (t + 1)]

            def xt_ap(t, h):
                c = 1024 * (t % 2) + 512 * h
                return xtc[t // 2][:, c:c + 512]

            def ltd_ap(u):
                return ltu[u][:, 0:128]

            def ltx_ap(u, i):
                return ltu[i][:, 128 * (u - i):128 * (u - i + 1)]

            # ---- DMA in: contiguous col-slice copies, big partition lines.
            # The DMA fabric serves ALL queued transfers concurrently
            # (~320 GB/s shared) and the Tile scheduler reorders dep-free
            # instructions, so priority needs REAL dependencies: x (which
            # gates block 0) + bt/wx0 go out ungated; every later wx/lt
            # tile's DMA sits behind a tiny gpsimd copy that writes into
            # that tile while reading a previous-wave tile (WAR chain), so
            # the bulk releases in ~1MB waves after x has landed.
            nc.scalar.dma_start(out=bt[:], in_=btr.ap()[:, :])
            nc.gpsimd.dma_start(out=wxu[0][:], in_=wxr.ap()[:, 0:1024])
            nc.sync.dma_start(out=xtc[0][:], in_=xtr.ap()[:, 0:2048])
            nc.gpsimd.dma_start(out=xtc[1][:], in_=xtr.ap()[:, 2048:4096])
            nc.sync.dma_start(out=xtc[2][:], in_=xtr.ap()[:, 4096:6144])
            nc.gpsimd.dma_start(out=xtc[3][:], in_=xtr.ap()[:, 6144:8192])

            def dma_wx(eng, u):
                eng.dma_start(out=wxu[u][:],
                              in_=wxr.ap()[:, 1024 * u:1024 * (u + 1)])

            def dma_lt(eng, i):
                eng.dma_start(out=ltu[i][:],
                              in_=ltr.ap()[:, LOFF[i]:LOFF[i] + M - 128 * i])

            # four gate chains, released wave by wave
            chains = [
                [xtc[0], wxu[1], wxu[3], wxu[5], wxu[7], wxu[9]],
                [xtc[1], ltu[0], ltu[2], ltu[4], ltu[6], ltu[8]],
                [xtc[2], wxu[2], wxu[4], wxu[6], wxu[8]],
                [xtc[3], ltu[1], ltu[3], ltu[5], ltu[7], ltu[9]],
            ]
            wx_set = {id(wxu[u]): u for u in range(NB)}
            lt_set = {id(ltu[i]): i for i in range(NB)}
            for w in range(1, 6):
                for ch in chains:
                    if w >= len(ch):
                        continue
                    prev, cur = ch[w - 1], ch[w]
                    nc.gpsimd.tensor_copy(out=cur[0:1, 0:2], in_=prev[0:1, 0:2])
                    if id(cur) in wx_set:
                        dma_wx(nc.gpsimd, wx_set[id(cur)])
                    else:
                        dma_lt(nc.gpsimd, lt_set[id(cur)])

            with (
                tc.tile_pool(name="pp_pool", bufs=3, space="PSUM") as pp_pool,
                tc.tile_pool(name="wu_pool", bufs=1, space="PSUM") as wu_pool,
                tc.tile_pool(name="ht0_pool", bufs=2) as ht0_pool,
                tc.tile_pool(name="ht1_pool", bufs=2) as ht1_pool,
                tc.tile_pool(name="dt_pool", bufs=2) as dt_pool,
                tc.tile_pool(name="y_pool", bufs=2) as y_pool,
            ):
                # PE warmup while the x DMA is in flight: ~4us of dummy
                # matmuls flips the HAM clock gate to 8/8 before real work,
                # so block 0's matmuls run at 2.4 GHz instead of 1.2.
                wup = sb.tile([128, 128], BF16, name="wup", tag="wup")
                nc.vector.memset(wup[:], 0.25)
                wps = wu_pool.tile([128, 128], F32, name="wps", tag="wps")
                for i in range(32):
                    nc.tensor.matmul(wps[:], wup[:], wup[:], start=True, stop=True)
                def mm_input(u, t_ps, t, h, start):
                    sl = slice(HALF * h, HALF * (h + 1))
                    nc.tensor.matmul(
                        t_ps[:, sl], wx_ap(t, u), xt_ap(t, h),
                        start=start, stop=False)

                def mm_cross(u, t_ps, i, h, last=False):
                    sl = slice(HALF * h, HALF * (h + 1))
                    nc.tensor.matmul(
                        t_ps[:, sl], ltx_ap(u, i),
                        hb[:, BC * i + HALF * h:BC * i + HALF * (h + 1)],
                        start=False, stop=last)

                def mm_iter(u, t_ps, rhs, h):
                    sl = slice(HALF * h, HALF * (h + 1))
                    nc.tensor.matmul(
                        t_ps[:, sl], ltd_ap(u), rhs[:, sl],
                        start=False, stop=False, skip_group_check=True)

                t_cur = pp_pool.tile([128, BC], F32, name="pp", tag="pp")
                for t in range(KT):
                    for h in range(2):
                        mm_input(0, t_cur, t, h, start=(t == 0))

                yhalf = []   # deferred sigmoid work: (ub, h)

                def emit_sigmoid(ub, h, ytiles):
                    if ub not in ytiles:
                        ytiles[ub] = y_pool.tile([128, BC], F32, name="y", tag="y")
                    yt = ytiles[ub]
                    sl = slice(HALF * h, HALF * (h + 1))
                    nc.scalar.activation(
                        yt[:, sl], hb[:, BC * ub + HALF * h:BC * ub + HALF * (h + 1)],
                        AF.Sigmoid)
                    return yt

                ytiles = {}
                for u in range(NB):
                    bcol = bt[:, u:u + 1]
                    last = u + 1 >= NB

                    # prefetch MM stream for block u+1, split into chunks that
                    # sandwich the iteration matmuls (keeps PE FIFO fed while
                    # the tanh chain runs, without delaying the chain)
                    pre = []
                    t_nxt = None
                    if not last:
                        t_nxt = pp_pool.tile([128, BC], F32, name="pp", tag="pp")
                        for t in range(KT):
                            for h in range(2):
                                pre.append(("in", t, h, t == 0))
                        for i in range(u):
                            for h in range(2):
                                pre.append(("x", i, h, False))

                    def emit_pre(n):
                        for _ in range(n):
                            if not pre:
                                return
                            kind, a, h, s = pre.pop(0)
                            if kind == "in":
                                mm_input(u + 1, t_nxt, a, h, start=s)
                            else:
                                mm_cross(u + 1, t_nxt, a, h)

                    # B. h0 = tanh(t + b)
                    ht0 = ht0_pool.tile([128, BC], BF16, name="ht0", tag="ht0")
                    for h in range(2):
                        sl = slice(HALF * h, HALF * (h + 1))
                        nc.scalar.activation(ht0[:, sl], t_cur[:, sl], AF.Tanh, bias=bcol)
                    if u == NB - 1:
                        # slot block NB-2's sigmoid halves into the chain
                        # bubbles (ACT idles while the iter matmuls run)
                        emit_sigmoid(NB - 2, 0, ytiles)

                    emit_pre(6)                 # ~1.3us of PE work before iter1
                    mm_iter(u, t_cur, ht0, 0)   # C
                    mm_iter(u, t_cur, ht0, 1)

                    # D. h1 = tanh(t + b)
                    ht1 = ht1_pool.tile([128, BC], BF16, name="ht1", tag="ht1")
                    for h in range(2):
                        sl = slice(HALF * h, HALF * (h + 1))
                        nc.scalar.activation(ht1[:, sl], t_cur[:, sl], AF.Tanh, bias=bcol)
                    if u == NB - 1:
                        emit_sigmoid(NB - 2, 1, ytiles)

                    # E. d = h1 - h0 (bf16 DVE)
                    dt = dt_pool.tile([128, BC], BF16, name="dt", tag="dt")
                    for h in range(2):
                        sl = slice(HALF * h, HALF * (h + 1))
                        nc.vector.tensor_tensor(
                            out=dt[:, sl], in0=ht1[:, sl], in1=ht0[:, sl], op=ALU.subtract)

                    emit_pre(5)                 # cover the h1+sub latency
                    mm_iter(u, t_cur, dt, 0)    # F
                    mm_iter(u, t_cur, dt, 1)

                    # G. h2 = tanh(t + b) -> final
                    for h in range(2):
                        sl = slice(HALF * h, HALF * (h + 1))
                        nc.scalar.activation(
                            hb[:, BC * u + HALF * h:BC * u + HALF * (h + 1)],
                            t_cur[:, sl], AF.Tanh, bias=bcol)
                        if u == NB - 1:
                            # final outputs right behind the last tanh halves
                            emit_sigmoid(NB - 1, h, ytiles)

                    emit_pre(len(pre))          # drain remaining prefetch
                    if not last:
                        # adjacent cross u+1 <- u (needs h2)
                        for h in range(2):
                            mm_cross(u + 1, t_nxt, u, h, last=(h == 1))
                        t_cur = t_nxt

                # DMA out
                y8 = ytiles[NB - 2]
                y9 = ytiles[NB - 1]
                nc.gpsimd.dma_start(out=yT.ap()[0:128, :], in_=y8[:])
                nc.gpsimd.dma_start(out=yT.ap()[128:256, 0:HALF], in_=y9[:, 0:HALF])
                nc.gpsimd.dma_start(out=yT.ap()[128:256, HALF:], in_=y9[:, HALF:])
    return nc


_nc_cache = None
BF16NP = ml_dtypes.bfloat16


def _prep(x, W, b):
    """Rearrange full inputs into per-core SBUF-layout arrays."""
    x = np.asarray(x, dtype=np.float32)
    W = np.asarray(W, dtype=np.float32)
    b = np.asarray(b, dtype=np.float32)

    # wxr: [128, 10240] — W[:, :IN].T block-major: cols = 1024u + 128t + c
    WxT = W[:, :IN].T.astype(BF16NP)                  # [IN, M]
    A = WxT.reshape(KT, 128, M)                       # [t, p, m]
    wxr = np.ascontiguousarray(np.concatenate(
        [A[:, :, 128 * u:128 * (u + 1)].transpose(1, 0, 2).reshape(128, -1)
         for u in range(NB)], axis=1))

    # ltr: [128, 7040] — strictly-lower L blocks, row-block i cols [128i:M)
    LT = np.triu(W[:, IN:].T, 1).astype(BF16NP)       # [M, M]
    ltr = np.ascontiguousarray(np.concatenate(
        [LT[128 * i:128 * (i + 1), 128 * i:] for i in range(NB)], axis=1))

    # btr: [128, NB]
    btr = np.ascontiguousarray(b.reshape(NB, 128).T)

    # xtr per core: [128, 8192], cols = 1024t + 512h + c
    xb = x.astype(BF16NP)
    xtrs = []
    for c in range(NCORES):
        xTc = np.ascontiguousarray(xb[c * BC:(c + 1) * BC].T)   # [IN, BC]
        arr = xTc.reshape(KT, 128, 2, HALF)                     # [t, p, h, c]
        xtrs.append(np.ascontiguousarray(
            arr.transpose(1, 0, 2, 3).reshape(128, -1)))
    return xtrs, wxr, ltr, btr


def kernel(x, W, b):
    global _nc_cache
    xtrs, wxr, ltr, btr = _prep(x, W, b)

    if _nc_cache is None:
        _nc_cache = _build()

    in_maps = [
        {"xtr": xtrs[c], "wxr": wxr, "ltr": ltr, "btr": btr}
        for c in range(NCORES)
    ]
    res = run_bass_kernel_spmd(_nc_cache, in_maps, list(range(NCORES)))
    y = np.concatenate(
        [np.ascontiguousarray(res.results[c]["yT"].T) for c in range(NCORES)], axis=0)
    return y


# revision 21
# speedup vs baseline: 2.0616x; 1.0422x over previous
"""Trainium2 Bass kernel for nn_DAG_72782515798738.

Math: node j (of M=1280) computes h_j = tanh(b_j + sum_{k<IN+j} W[j,k]*state_k)
over states = [x (IN=1024), h (M)], batch B=8192. Output y = sigmoid(h[HID:]).

Strategy: data-parallel over batch (8 cores x 1024 rows). Per core, the node
recurrence is solved block-by-block (10 blocks of 128 nodes, NODE-major tiles
[node, batch]). All matmuls run in bf16 (PSUM accumulates fp32); the 2e-2
correctness gate leaves ample room (measured ~6e-3). Per block the
pre-activation accumulator t lives in PSUM: prefetched input/cross matmuls
build t = p, then two fixed-point refinements accumulate in place:
  h0 = tanh(t+b);  t += Ld@h0;  h1 = tanh(t+b);  t += Ld@(h1-h0);  h2 = tanh(t+b)
so the only non-PE work on the chain is the tanh (ACT) and one bf16 DVE sub.

All inputs are pre-rearranged on the host into the exact SBUF tile layouts so
every DMA is a fully-contiguous col-slice copy (8-16KB per partition line).
Prefetch matmuls for block u+1 are interleaved around the iteration matmuls
of block u so the PE FIFO never stalls on the activation chain.
"""
import numpy as np
import ml_dtypes

import concourse.bass as bass
import concourse.mybir as mybir
from concourse.tile import TileContext
from concourse.vector_clock import ScopedClock
from concourse.bass_utils import run_bass_kernel_spmd

F32 = mybir.dt.float32
BF16 = mybir.dt.bfloat16
AF = mybir.ActivationFunctionType
ALU = mybir.AluOpType

IN, HID, OUT = 1024, 1024, 256
M = HID + OUT          # 1280 computed nodes
B = 8192
NCORES = 8
BC = B // NCORES       # 1024 batch rows per core
K = 128                # node block size
NB = M // K            # 10 blocks
KT = IN // 128         # 8 contraction tiles for the input matmul
HALF = BC // 2         # 512

# lt packing offsets: block i occupies cols [LOFF[i], LOFF[i] + M - 128*i)
LOFF = [0]
for _i in range(1, NB):
    LOFF.append(LOFF[-1] + (M - 128 * (_i - 1)))
LTOT = LOFF[-1] + (M - 128 * (NB - 1))  # 7040

# wx packing: group g0 = block 0 (ktile t at 128t), g1 = blocks 1-4
# (ktile t at 1024 + 512t), g2 = blocks 5-9 (ktile t at 5120 + 640t)
WXTOT = KT * M  # 10240

_wsplit_ctr = [0]


class _TileContextFix(TileContext):
    """This walrus build accepts only ONE embedded sem-wait per instruction;
    split extra waits onto single-wait NOPs, and same for the exit drain."""

    def _add_instruction(self, inst):
        si = getattr(inst, "sync_info", None)
        if si is not None and si.on_wait is not None and len(si.on_wait) > 1:
            waits = list(si.on_wait)
            for w in waits[:-1]:
                _wsplit_ctr[0] += 1
                nop = mybir.InstNoOp(name=f"wsplit_{_wsplit_ctr[0]}", ins=[], outs=[])
                nop.engine = inst.engine
                nop.sync_info = mybir.SyncInfo(on_wait=[w], on_update=[])
                super()._add_instruction(nop)
            si.on_wait = waits[-1:]
        super()._add_instruction(inst)

    def _drain_and_barrier(self, tick_clock, wait_clock):
        nc = self.nc
        probe = nc.sync.nop(nofuse=True, hint="exit_wait_carrier")
        wait_clock.add_sem_waits(probe.ins, ScopedClock({None: tick_clock.global_clock}))
        si = probe.ins.sync_info
        waits = list(si.on_wait) if si is not None and si.on_wait else []
        if len(waits) > 1:
            si.on_wait = waits[:1]
            for w in waits[1:]:
                n2 = nc.sync.nop(nofuse=True, hint="exit_wait_carrier")
                if n2.ins.sync_info is None:
                    n2.ins.sync_info = mybir.SyncInfo(on_wait=[w], on_update=[])
                else:
                    n2.ins.sync_info.on_wait = [w]
        nc.sync.drain()
        nc.all_engine_barrier()
        assert self.sems is not None
        popped = nc._tile_sem_poison_stack.pop()
        assert popped is self._sem_poison
        nc.clear_and_free_semaphores(list(self.sems.allocated().values()))
        nc.all_engine_barrier()


def _build():
    nc = bass.Bass("TRN2", target_bir_lowering=False, debug=False, num_devices=NCORES)

    # host-rearranged inputs, already in SBUF tile layout
    xtr = nc.dram_tensor("xtr", [128, 2 * KT * HALF], BF16, kind="ExternalInput")
    wxr = nc.dram_tensor("wxr", [128, WXTOT], BF16, kind="ExternalInput")
    ltr = nc.dram_tensor("ltr", [128, LTOT], BF16, kind="ExternalInput")
    btr = nc.dram_tensor("btr", [128, NB], F32, kind="ExternalInput")
    yT = nc.dram_tensor("yT", [OUT, BC], F32, kind="ExternalOutput")

    with _TileContextFix(nc) as tc:
        with (
            tc.tile_pool(name="sb", bufs=1) as sb,
        ):
            # persistent SBUF tiles. One tile per independently-DMA'd chunk:
            # Tile makes any reader wait on ALL writers of a tile, so a
            # consumer must share a tile only with the DMA that feeds it.
            xtc = [sb.tile([128, 2048], BF16, name=f"xt{j}", tag=f"xt{j}")
                   for j in range(4)]                     # ktile pair j
            wxu = [sb.tile([128, KT * 128], BF16, name=f"wx{u}", tag=f"wx{u}")
                   for u in range(NB)]                    # input weights, block u
            ltu = [sb.tile([128, M - 128 * i], BF16, name=f"lt{i}", tag=f"lt{i}")
                   for i in range(NB)]                    # L rows of block i
            hb = sb.tile([128, NB * BC], BF16, name="hb", tag="hb")
            bt = sb.tile([128, NB], F32, name="bt", tag="bt")

            def wx_ap(t, u):
                return wxu[u][:, 128 * t:128 * (t + 1)]

            def xt_ap(t, h):
                c = 1024 * (t % 2) + 512 * h
                return xtc[t // 2][:, c:c + 512]

            def ltd_ap(u):
                return ltu[u][:, 0:128]

            def ltx_ap(u, i):
                return ltu[i][:, 128 * (u - i):128 * (u - i + 1)]

            # ---- DMA in: contiguous col-slice copies, big partition lines.
            # The DMA fabric serves ALL queued transfers concurrently
            # (~320 GB/s shared) and the Tile scheduler reorders dep-free
            # instructions, so priority needs REAL dependencies: x (which
            # gates block 0) + bt/wx0 go out ungated; every later wx/lt
            # tile's DMA sits behind a tiny gpsimd copy that writes into
            # that tile while reading a previous-wave tile (WAR chain), so
            # the bulk releases in ~1MB waves after x has landed.
            nc.scalar.dma_start(out=bt[:], in_=btr.ap()[:, :])
            nc.gpsimd.dma_start(out=wxu[0][:], in_=wxr.ap()[:, 0:1024])
            nc.sync.dma_start(out=xtc[0][:], in_=xtr.ap()[:, 0:2048])
            nc.gpsimd.dma_start(out=xtc[1][:], in_=xtr.ap()[:, 2048:4096])
            nc.sync.dma_start(out=xtc[2][:], in_=xtr.ap()[:, 4096:6144])
            nc.gpsimd.dma_start(out=xtc[3][:], in_=xtr.ap()[:, 6144:8192])

            def dma_wx(eng, u):
                eng.dma_start(out=wxu[u][:],
                              in_=wxr.ap()[:, 1024 * u:1024 * (u + 1)])

            def dma_lt(eng, i):
                eng.dma_start(out=ltu[i][:],
                              in_=ltr.ap()[:, LOFF[i]:LOFF[i] + M - 128 * i])

            # four gate chains, released wave by wave
            chains = [
                [xtc[0], wxu[1], wxu[3], wxu[5], wxu[7], wxu[9]],
                [xtc[1], ltu[0], ltu[2], ltu[4], ltu[6], ltu[8]],
                [xtc[2], wxu[2], wxu[4], wxu[6], wxu[8]],
                [xtc[3], ltu[1], ltu[3], ltu[5], ltu[7], ltu[9]],
            ]
            wx_set = {id(wxu[u]): u for u in range(NB)}
            lt_set = {id(ltu[i]): i for i in range(NB)}
            for w in range(1, 6):
                for ch in chains:
                    if w >= len(ch):
                        continue
                    prev, cur = ch[w - 1], ch[w]
                    nc.gpsimd.tensor_copy(out=cur[0:1, 0:2], in_=prev[0:1, 0:2])
                    if id(cur) in wx_set:
                        dma_wx(nc.gpsimd, wx_set[id(cur)])
                    else:
                        dma_lt(nc.gpsimd, lt_set[id(cur)])

            with (
                tc.tile_pool(name="pp_pool", bufs=3, space="PSUM") as pp_pool,
                tc.tile_pool(name="wu_pool", bufs=1, space="PSUM") as wu_pool,
                tc.tile_pool(name="ht0_pool", bufs=2) as ht0_pool,
                tc.tile_pool(name="ht1_pool", bufs=2) as ht1_pool,
                tc.tile_pool(name="dt_pool", bufs=2) as dt_pool,
                tc.tile_pool(name="y_pool", bufs=2) as y_pool,
            ):
                # PE warmup while the x DMA is in flight: ~6us of dummy
                # matmuls flips the HAM clock gate to 8/8 before real work,
                # so block 0's matmuls run at 2.4 GHz instead of 1.2. The
                # dummy activation pulls the ~2.7us ACT table load (inserted
                # by walrus before the first ACTIVATE) off the critical path.
                wup = sb.tile([128, 128], BF16, name="wup", tag="wup")
                wup2 = sb.tile([128, 128], BF16, name="wup2", tag="wup2")
                nc.vector.memset(wup[:], 0.25)
                nc.scalar.activation(wup2[:], wup[:], AF.Tanh)
                wps = wu_pool.tile([128, 128], F32, name="wps", tag="wps")
                for i in range(60):
                    nc.tensor.matmul(wps[:], wup[:], wup[:], start=True, stop=True)
                def mm_input(u, t_ps, t, h, start):
                    sl = slice(HALF * h, HALF * (h + 1))
                    nc.tensor.matmul(
                        t_ps[:, sl], wx_ap(t, u), xt_ap(t, h),
                        start=start, stop=False)

                def mm_cross(u, t_ps, i, h, last=False):
                    sl = slice(HALF * h, HALF * (h + 1))
                    nc.tensor.matmul(
                        t_ps[:, sl], ltx_ap(u, i),
                        hb[:, BC * i + HALF * h:BC * i + HALF * (h + 1)],
                        start=False, stop=last)

                def mm_iter(u, t_ps, rhs, h):
                    sl = slice(HALF * h, HALF * (h + 1))
                    nc.tensor.matmul(
                        t_ps[:, sl], ltd_ap(u), rhs[:, sl],
                        start=False, stop=False, skip_group_check=True)

                t_cur = pp_pool.tile([128, BC], F32, name="pp", tag="pp")
                for t in range(KT):
                    for h in range(2):
                        mm_input(0, t_cur, t, h, start=(t == 0))

                yhalf = []   # deferred sigmoid work: (ub, h)

                def emit_sigmoid(ub, h, ytiles):
                    if ub not in ytiles:
                        ytiles[ub] = y_pool.tile([128, BC], F32, name="y", tag="y")
                    yt = ytiles[ub]
                    sl = slice(HALF * h, HALF * (h + 1))
                    nc.scalar.activation(
                        yt[:, sl], hb[:, BC * ub + HALF * h:BC * ub + HALF * (h + 1)],
                        AF.Sigmoid)
                    return yt

                ytiles = {}
                for u in range(NB):
                    bcol = bt[:, u:u + 1]
                    last = u + 1 >= NB

                    # prefetch MM stream for block u+1, split into chunks that
                    # sandwich the iteration matmuls (keeps PE FIFO fed while
                    # the tanh chain runs, without delaying the chain)
                    pre = []
                    t_nxt = None
                    if not last:
                        t_nxt = pp_pool.tile([128, BC], F32, name="pp", tag="pp")
                        for t in range(KT):
                            for h in range(2):
                                pre.append(("in", t, h, t == 0))
                        for i in range(u):
                            for h in range(2):
                                pre.append(("x", i, h, False))

                    def emit_pre(n):
                        for _ in range(n):
                            if not pre:
                                return
                            kind, a, h, s = pre.pop(0)
                            if kind == "in":
                                mm_input(u + 1, t_nxt, a, h, start=s)
                            else:
                                mm_cross(u + 1, t_nxt, a, h)

                    # B. h0 = tanh(t + b)
                    ht0 = ht0_pool.tile([128, BC], BF16, name="ht0", tag="ht0")
                    for h in range(2):
                        sl = slice(HALF * h, HALF * (h + 1))
                        nc.scalar.activation(ht0[:, sl], t_cur[:, sl], AF.Tanh, bias=bcol)
                    if u == NB - 1:
                        # slot block NB-2's sigmoid halves into the chain
                        # bubbles (ACT idles while the iter matmuls run)
                        emit_sigmoid(NB - 2, 0, ytiles)

                    emit_pre(6)                 # ~1.3us of PE work before iter1
                    mm_iter(u, t_cur, ht0, 0)   # C
                    mm_iter(u, t_cur, ht0, 1)

                    # D. h1 = tanh(t + b)
                    ht1 = ht1_pool.tile([128, BC], BF16, name="ht1", tag="ht1")
                    for h in range(2):
                        sl = slice(HALF * h, HALF * (h + 1))
                        nc.scalar.activation(ht1[:, sl], t_cur[:, sl], AF.Tanh, bias=bcol)
                    if u == NB - 1:
                        emit_sigmoid(NB - 2, 1, ytiles)

                    # E. d = h1 - h0 (bf16 DVE)
                    dt = dt_pool.tile([128, BC], BF16, name="dt", tag="dt")
                    for h in range(2):
                        sl = slice(HALF * h, HALF * (h + 1))
                        nc.vector.tensor_tensor(
                            out=dt[:, sl], in0=ht1[:, sl], in1=ht0[:, sl], op=ALU.subtract)

                    emit_pre(5)                 # cover the h1+sub latency
                    mm_iter(u, t_cur, dt, 0)    # F
                    mm_iter(u, t_cur, dt, 1)

                    # G. h2 = tanh(t + b) -> final
                    for h in range(2):
                        sl = slice(HALF * h, HALF * (h + 1))
                        nc.scalar.activation(
                            hb[:, BC * u + HALF * h:BC * u + HALF * (h + 1)],
                            t_cur[:, sl], AF.Tanh, bias=bcol)
                        if u == NB - 1:
                            # final outputs right behind the last tanh halves
                            emit_sigmoid(NB - 1, h, ytiles)

                    emit_pre(len(pre))          # drain remaining prefetch
                    if not last:
                        # adjacent cross u+1 <- u (needs h2)
                        for h in range(2):
                            mm_cross(u + 1, t_nxt, u, h, last=(h == 1))
                        t_cur = t_nxt

                # DMA out (sync queue: keeps the SWDGE ring empty at exit so
                # its final drain is cheap)
                y8 = ytiles[NB - 2]
                y9 = ytiles[NB - 1]
                nc.sync.dma_start(out=yT.ap()[0:128, :], in_=y8[:])
                nc.sync.dma_start(out=yT.ap()[128:256, 0:HALF], in_=y9[:, 0:HALF])
                nc.sync.dma_start(out=yT.ap()[128:256, HALF:], in_=y9[:, HALF:])
    return nc


_nc_cache = None
BF16NP = ml_dtypes.bfloat16


def _prep(x, W, b):
    """Rearrange full inputs into per-core SBUF-layout arrays."""
    x = np.asarray(x, dtype=np.float32)
    W = np.asarray(W, dtype=np.float32)
    b = np.asarray(b, dtype=np.float32)

    # wxr: [128, 10240] — W[:, :IN].T block-major: cols = 1024u + 128t + c
    WxT = W[:, :IN].T.astype(BF16NP)                  # [IN, M]
    A = WxT.reshape(KT, 128, M)                       # [t, p, m]
    wxr = np.ascontiguousarray(np.concatenate(
        [A[:, :, 128 * u:128 * (u + 1)].transpose(1, 0, 2).reshape(128, -1)
         for u in range(NB)], axis=1))

    # ltr: [128, 7040] — strictly-lower L blocks, row-block i cols [128i:M)
    LT = np.triu(W[:, IN:].T, 1).astype(BF16NP)       # [M, M]
    ltr = np.ascontiguousarray(np.concatenate(
        [LT[128 * i:128 * (i + 1), 128 * i:] for i in range(NB)], axis=1))

    # btr: [128, NB]
    btr = np.ascontiguousarray(b.reshape(NB, 128).T)

    # xtr per core: [128, 8192], cols = 1024t + 512h + c
    xb = x.astype(BF16NP)
    xtrs = []
    for c in range(NCORES):
        xTc = np.ascontiguousarray(xb[c * BC:(c + 1) * BC].T)   # [IN, BC]
        arr = xTc.reshape(KT, 128, 2, HALF)                     # [t, p, h, c]
        xtrs.append(np.ascontiguousarray(
            arr.transpose(1, 0, 2, 3).reshape(128, -1)))
    return xtrs, wxr, ltr, btr


def kernel(x, W, b):
    global _nc_cache
    xtrs, wxr, ltr, btr = _prep(x, W, b)

    if _nc_cache is None:
        _nc_cache = _build()

    in_maps = [
        {"xtr": xtrs[c], "wxr": wxr, "ltr": ltr, "btr": btr}
        for c in range(NCORES)
    ]
    res = run_bass_kernel_spmd(_nc_cache, in_maps, list(range(NCORES)))
    y = np.concatenate(
        [np.ascontiguousarray(res.results[c]["yT"].T) for c in range(NCORES)], axis=0)
    return y
